# revision 1
# baseline (speedup 1.0000x reference)
"""DeepSeek-MLA attention block on 8 Trainium2 NeuronCores.

Sharding: tensor-parallel over heads (16 heads / 8 cores = 2 heads per core).
All per-head projections (k/v compressor, q_a, q_gate, o_v) are head-local;
each core computes a partial output through its slice of Wo and the host sums
the 8 partials.

Device layout notes:
  - Everything token-major work happens transposed: activations live as
    [d, token] tiles so the d_model contraction sits on the partition dim.
  - RoPE is folded into the compression matmuls: for a per-token rotation
    x' = c .* x + s .* (P x) (P the fixed rotate-half permutation), any
    projection W satisfies  x'^T W = (c .* x)^T W + (s .* x)^T W_rot  with
    W_rot[e,:] = W[e+32,:] (e<32), -W[e-32,:] (32<=e<64), 0 (e>=64) —
    using sin's 32-periodicity. So no partition-moving rope stage at all.
  - Softmax runs unnormalized: exp(scores/8) with the denominator obtained by
    prepending a ones-column to v_c (row 0 of the out_c accumulation is then
    sum_k exp; a zero row in the padded Wov cancels it in the uplift).
    Normalization happens at the gating step: fast-approx reciprocal of the
    denominator row, broadcast across partitions on the idle GPSIMD engine.
  - Matmul operands are bf16 (fp32 matmul is 4x slower on TRN2); PSUM
    accumulation stays fp32, the softmax denominator/reciprocal stay fp32,
    and the final output is written fp32.
"""

import numpy as np
import ml_dtypes

HIDDEN = 2048
N_HEADS = 16
HEAD_DIM = 128
ROPE_DIM = 64
RANK = 64
BASE = 10000.0
N_CORES = 8
H_LOCAL = 2  # heads per core

BF16 = ml_dtypes.bfloat16


def _build_nc(S: int, CH: int = 512):
    """Build the SPMD Bass program for one core (B=2 fixed, seq len S)."""
    import concourse.bacc as bacc
    import concourse.tile as tile
    from concourse import mybir
    from contextlib import ExitStack

    f32 = mybir.dt.float32
    bf16 = mybir.dt.bfloat16
    AF = mybir.ActivationFunctionType

    B = 2
    T = B * S
    PT = HIDDEN // 128          # 16 d_model partition tiles
    KT = S // 128               # k-token tiles per pass
    NCH = S // CH               # token chunks per pass (phase A)
    QC = S // CH                # q chunks per pass (phase B)
    R1 = RANK + 1               # out_c rows incl. denominator row 0

    nc = bacc.Bacc("TRN2", target_bir_lowering=False, debug=False)

    ht = nc.dram_tensor("ht", [HIDDEN, T], bf16, kind="ExternalInput")
    wqkv = nc.dram_tensor("wqkv", [PT, 128, 768], bf16, kind="ExternalInput")
    wo_d = nc.dram_tensor("wo", [H_LOCAL, 128, HIDDEN], bf16, kind="ExternalInput")
    wkc_d = nc.dram_tensor("wkc", [128, RANK], bf16, kind="ExternalInput")
    wkcr_d = nc.dram_tensor("wkcr", [128, RANK], bf16, kind="ExternalInput")
    wvc_d = nc.dram_tensor("wvc", [128, RANK], bf16, kind="ExternalInput")
    wqa_d = nc.dram_tensor("wqa", [128, RANK], bf16, kind="ExternalInput")
    wqar_d = nc.dram_tensor("wqar", [128, RANK], bf16, kind="ExternalInput")
    wqg_d = nc.dram_tensor("wqg", [128, 128], bf16, kind="ExternalInput")
    wqgr_d = nc.dram_tensor("wqgr", [128, 128], bf16, kind="ExternalInput")
    wovp_d = nc.dram_tensor("wovp", [R1, 128], bf16, kind="ExternalInput")
    cext_d = nc.dram_tensor("cext", [128, T], bf16, kind="ExternalInput")
    sext_d = nc.dram_tensor("sext", [128, T], bf16, kind="ExternalInput")
    out_d = nc.dram_tensor("out", [T, HIDDEN], f32, kind="ExternalOutput")

    with tile.TileContext(nc) as tc, ExitStack() as ctx:
        const = ctx.enter_context(tc.tile_pool(name="const", bufs=1))
        hpool = ctx.enter_context(tc.tile_pool(name="hp", bufs=2))
        qkvpool = ctx.enter_context(tc.tile_pool(name="qkvp", bufs=2))
        xpool = ctx.enter_context(tc.tile_pool(name="xp", bufs=1))
        cpool = ctx.enter_context(tc.tile_pool(name="cp", bufs=1))
        epool = ctx.enter_context(tc.tile_pool(name="ep", bufs=2))
        mpool = ctx.enter_context(tc.tile_pool(name="mp", bufs=2))
        opool = ctx.enter_context(tc.tile_pool(name="op", bufs=4))
        psum = ctx.enter_context(tc.tile_pool(name="ps", bufs=1, space="PSUM"))

        # ---- constants ----
        w_qkv = const.tile([128, PT, 768], bf16, name="wqkv", tag="wqkv")
        for p in range(PT):
            nc.sync.dma_start(out=w_qkv[:, p, :], in_=wqkv[p])
        w_o = const.tile([128, H_LOCAL, HIDDEN], bf16, name="wo", tag="wo")
        for h in range(H_LOCAL):
            nc.sync.dma_start(out=w_o[:, h, :], in_=wo_d[h])
        sm = {}
        for name, d, shp in [
            ("wkc", wkc_d, [128, RANK]), ("wkcr", wkcr_d, [128, RANK]),
            ("wvc", wvc_d, [128, RANK]), ("wqa", wqa_d, [128, RANK]),
            ("wqar", wqar_d, [128, RANK]), ("wqg", wqg_d, [128, 128]),
            ("wqgr", wqgr_d, [128, 128]), ("wovp", wovp_d, [R1, 128]),
        ]:
            t = const.tile(shp, bf16, tag=name)
            nc.sync.dma_start(out=t[:], in_=d[:])
            sm[name] = t
        from concourse import library_config
        nc.gpsimd.load_library(library_config.attn)

        for b in range(B):
            base = b * S
            # per-pass persistent compressed tensors; kc/qa hold head h in
            # partitions [64h, 64h+64) so score matmuls row-pack both heads
            # into disjoint PE row groups (concurrent execution)
            kc_all = cpool.tile([128, S], bf16, name="kca", tag="kca", bufs=2)
            qa_all = cpool.tile([128, S], bf16, name="qaa", tag="qaa", bufs=2)
            sg = [cpool.tile([128, S], bf16, name=f"sg{h}", tag=f"sg{h}") for h in range(H_LOCAL)]
            qgs = [cpool.tile([128, S], bf16, name=f"qg{h}", tag=f"qg{h}") for h in range(H_LOCAL)]
            vc = [cpool.tile([128, KT, R1], bf16, name=f"vc{h}", tag=f"vc{h}", bufs=2) for h in range(H_LOCAL)]
            for h in range(H_LOCAL):
                nc.vector.memset(vc[h][:, :, 0:1], 1.0)
            cext = cpool.tile([128, S], bf16, name="cext", tag="cext")
            nc.sync.dma_start(out=cext[:], in_=cext_d[:, base:base + S])
            sext = cpool.tile([128, S], bf16, name="sext", tag="sext")
            nc.sync.dma_start(out=sext[:], in_=sext_d[:, base:base + S])

            # ---------------- Phase A: projections + compressions ------------
            for c in range(NCH):
                tcol = base + c * CH
                cc = c * CH
                h_t = hpool.tile([128, PT, CH], bf16, name="hch", tag="hch")
                for p in range(PT):
                    nc.sync.dma_start(
                        out=h_t[:, p, :],
                        in_=ht[p * 128:(p + 1) * 128, tcol:tcol + CH],
                    )
                qkv_t = []
                for j in range(6):
                    ps = psum.tile([128, CH], f32, name=f"qkv_{j % 2}", tag=f"qkv_{j % 2}")
                    for p in range(PT):
                        nc.tensor.matmul(
                            ps,
                            w_qkv[:, p, j * 128:(j + 1) * 128],
                            h_t[:, p, :],
                            start=(p == 0),
                            stop=(p == PT - 1),
                        )
                    sb = qkvpool.tile([128, CH], bf16, name=f"qkvsb{j}", tag=f"qkvsb{j}")
                    nc.vector.tensor_copy(sb, ps)
                    qkv_t.append(sb)

                csl = cext[:, cc:cc + CH]
                ssl = sext[:, cc:cc + CH]
                cmp_i = 0
                q_c, q_s, k_cx, k_sx = [], [], [], []
                for h in range(H_LOCAL):
                    jq, jk = h, 2 + h
                    t = xpool.tile([128, CH], bf16, name=f"xqc{h}", tag=f"xqc{h}")
                    nc.vector.tensor_mul(t, qkv_t[jq], csl); q_c.append(t)
                    t = xpool.tile([128, CH], bf16, name=f"xqs{h}", tag=f"xqs{h}")
                    nc.vector.tensor_mul(t, qkv_t[jq], ssl); q_s.append(t)
                    t = xpool.tile([128, CH], bf16, name=f"xkc{h}", tag=f"xkc{h}")
                    nc.vector.tensor_mul(t, qkv_t[jk], csl); k_cx.append(t)
                    t = xpool.tile([128, CH], bf16, name=f"xks{h}", tag=f"xks{h}")
                    nc.vector.tensor_mul(t, qkv_t[jk], ssl); k_sx.append(t)

                # kc / qa for both heads col-packed into one [128, CH] psum:
                # head h lands in psum partitions [64h, 64h+64)
                for dst, wc, wr, xc_, xs_ in (
                    (kc_all, "wkc", "wkcr", k_cx, k_sx),
                    (qa_all, "wqa", "wqar", q_c, q_s),
                ):
                    ps = psum.tile([128, CH], f32, name=f"cmp_{cmp_i % 2}", tag=f"cmp_{cmp_i % 2}"); cmp_i += 1
                    for h in range(H_LOCAL):
                        tp = (0, 64 * h) if h else None
                        nc.tensor.matmul(ps[64 * h:64 * h + 64, :], sm[wc], xc_[h],
                                         start=True, stop=False, tile_position=tp)
                        nc.tensor.matmul(ps[64 * h:64 * h + 64, :], sm[wr], xs_[h],
                                         start=False, stop=True, tile_position=tp)
                    nc.vector.tensor_copy(dst[:, cc:cc + CH], ps)

                for h in range(H_LOCAL):
                    jv = 4 + h
                    ps = psum.tile([128, CH], f32, name=f"cmp_{cmp_i % 2}", tag=f"cmp_{cmp_i % 2}"); cmp_i += 1
                    nc.tensor.matmul(ps, sm["wqg"], q_c[h], start=True, stop=False)
                    nc.tensor.matmul(ps, sm["wqgr"], q_s[h], start=False, stop=True)
                    # silu(x) = x * sigmoid(x); sigmoid fused into the
                    # psum->sbuf drain, raw q_gate kept for the multiply
                    nc.scalar.activation(sg[h][:, cc:cc + CH], ps, AF.Sigmoid)
                    nc.vector.tensor_copy(qgs[h][:, cc:cc + CH], ps)

                    for tt in range(CH // 128):
                        ps = psum.tile([128, RANK], f32, name=f"cmp_{cmp_i % 2}", tag=f"cmp_{cmp_i % 2}"); cmp_i += 1
                        nc.tensor.matmul(
                            ps,
                            qkv_t[jv][:, tt * 128:(tt + 1) * 128],
                            sm["wvc"],
                            start=True,
                            stop=True,
                        )
                        nc.vector.tensor_copy(
                            vc[h][:, c * (CH // 128) + tt, 1:R1], ps
                        )

            # ---------------- Phase B: attention + gating + Wo ---------------
            for qc in range(QC):
                qcc = qc * CH
                # both heads' score matmuls issue back-to-back: head h's
                # operands live in partitions [64h, 64h+64) so the PE runs
                # them concurrently in disjoint row groups
                exps = [epool.tile([128, 4, CH], bf16, name=f"exps{h}", tag=f"exps{h}", bufs=1)
                        for h in range(H_LOCAL)]
                oc_ps = [psum.tile([R1, CH], f32, name=f"oc{h}", tag=f"oc{h}")
                         for h in range(H_LOCAL)]
                for kt in range(KT):
                    # emit both heads' score matmuls adjacently so the PE can
                    # run them concurrently in disjoint row groups
                    s_ps = []
                    for h in range(H_LOCAL):
                        # in the last pass the phase-A qkv banks are free for
                        # good (no next pass), so double-buffer scores there
                        stag = f"s_{h}" if (b < B - 1 or kt % 2 == 0) else f"qkv_{h}"
                        sp = psum.tile([128, CH], f32, name=f"s_{h}", tag=stag)
                        nc.tensor.matmul(
                            sp,
                            kc_all[64 * h:64 * h + 64, kt * 128:(kt + 1) * 128],
                            qa_all[64 * h:64 * h + 64, qcc:qcc + CH],
                            start=True,
                            stop=True,
                            tile_position=(64 * h, 0),
                        )
                        s_ps.append(sp)
                    for h in range(H_LOCAL):
                        nc.scalar.activation(
                            exps[h][:, kt % 4, :], s_ps[h], AF.Exp,
                            scale=float(1.0 / np.sqrt(RANK))
                        )
                    for h in range(H_LOCAL):
                        nc.tensor.matmul(
                            oc_ps[h],
                            vc[h][:, kt, :],
                            exps[h][:, kt % 4, :],
                            start=(kt == 0),
                            stop=(kt == KT - 1),
                        )
                gated = []
                for h in range(H_LOCAL):
                    den = mpool.tile([1, CH], f32, name=f"den{h}", tag=f"den{h}", bufs=1)
                    nc.vector.tensor_copy(den, oc_ps[h][0:1, :])
                    oc_sb = mpool.tile([R1, CH], bf16, name=f"ocsb{h}", tag=f"ocsb{h}")
                    nc.vector.tensor_copy(oc_sb, oc_ps[h])
                    rdet = mpool.tile([1, CH], f32, name=f"rdet{h}", tag=f"rdet{h}", bufs=1)
                    nc.vector.reciprocal_approx_fast(rdet, den)
                    # broadcast 1/denom across partitions on the (idle)
                    # gpsimd engine, freeing the PE and a psum bank
                    bcast = mpool.tile([128, CH], f32, name="bcsb", tag="bcsb")
                    nc.gpsimd.partition_broadcast(bcast, rdet)
                    up_ps = psum.tile([128, CH], f32, name="up", tag="cmp_0")
                    nc.tensor.matmul(up_ps, sm["wovp"], oc_sb, start=True, stop=True)
                    t1 = mpool.tile([128, CH], f32, name="t1", tag="t1", bufs=1)
                    nc.vector.tensor_mul(t1, up_ps, sg[h][:, qcc:qcc + CH])
                    t2 = mpool.tile([128, CH], f32, name="t2", tag="t2", bufs=1)
                    nc.vector.tensor_mul(t2, t1, qgs[h][:, qcc:qcc + CH])
                    g = mpool.tile([128, CH], bf16, name=f"gated{h}", tag=f"gated{h}")
                    nc.vector.tensor_mul(g, t2, bcast)
                    gated.append(g)

                wo_i = 0
                for tt in range(CH // 128):
                    for n in range(HIDDEN // 512):
                        wo_ps = psum.tile([128, 512], f32, name=f"cmp_{wo_i % 2}", tag=f"cmp_{wo_i % 2}"); wo_i += 1
                        for h in range(H_LOCAL):
                            nc.tensor.matmul(
                                wo_ps,
                                gated[h][:, tt * 128:(tt + 1) * 128],
                                w_o[:, h, n * 512:(n + 1) * 512],
                                start=(h == 0),
                                stop=(h == H_LOCAL - 1),
                            )
                        ost = opool.tile([128, 512], f32, name="ost", tag="ost")
                        nc.vector.tensor_copy(ost, wo_ps)
                        r0 = base + qcc + tt * 128
                        nc.sync.dma_start(
                            out=out_d[r0:r0 + 128, n * 512:(n + 1) * 512], in_=ost
                        )
    nc.compile()
    return nc


def _rot_w(w):
    """Fold rotate-half into a projection matrix (see module docstring)."""
    r = np.zeros_like(w)
    r[0:32] = w[32:64]
    r[32:64] = -w[0:32]
    return r


def _host_inputs(hidden_states, position_ids, Wq, Wk, Wv, Wkc, Wvc, Wqa, Wqg,
                 Wov, Wo, S):
    """Build the 8 per-core input maps (all device arrays bf16)."""
    B = 2
    T = B * S
    h = np.asarray(hidden_states, dtype=np.float32).reshape(T, HIDDEN)
    ht = np.ascontiguousarray(h.T).astype(BF16)

    pos = np.asarray(position_ids).reshape(-1).astype(np.float64)
    pos = np.concatenate([pos] * B)  # token order is [b0 tokens, b1 tokens]
    inv_freq = 1.0 / (BASE ** (np.arange(0, ROPE_DIM, 2, dtype=np.float64) / ROPE_DIM))
    freqs = np.outer(pos, inv_freq)                       # [T, 32]
    emb = np.concatenate([freqs, freqs], axis=1)          # [T, 64]
    cext = np.ones((128, T), dtype=np.float32)
    sext = np.zeros((128, T), dtype=np.float32)
    cext[0:ROPE_DIM] = np.cos(emb).T
    sext[0:ROPE_DIM] = np.sin(emb).T

    Wkc = np.asarray(Wkc, np.float32); Wvc = np.asarray(Wvc, np.float32)
    Wqa = np.asarray(Wqa, np.float32); Wqg = np.asarray(Wqg, np.float32)
    Wov = np.asarray(Wov, np.float32)
    wovp = np.concatenate([np.zeros((1, 128), np.float32), Wov], axis=0)

    shared = {
        "ht": ht,
        "wkc": Wkc.astype(BF16), "wkcr": _rot_w(Wkc).astype(BF16),
        "wvc": Wvc.astype(BF16),
        "wqa": Wqa.astype(BF16), "wqar": _rot_w(Wqa).astype(BF16),
        "wqg": Wqg.astype(BF16), "wqgr": _rot_w(Wqg).astype(BF16),
        "wovp": wovp.astype(BF16),
        "cext": cext.astype(BF16), "sext": sext.astype(BF16),
    }

    Wq = np.asarray(Wq, np.float32); Wk = np.asarray(Wk, np.float32)
    Wv = np.asarray(Wv, np.float32); Wo = np.asarray(Wo, np.float32)
    in_maps = []
    for c in range(N_CORES):
        cols = slice(c * 256, (c + 1) * 256)
        wbig = np.concatenate([Wq[:, cols], Wk[:, cols], Wv[:, cols]], axis=1)
        m = dict(shared)
        m["wqkv"] = np.ascontiguousarray(wbig.reshape(HIDDEN // 128, 128, 768)).astype(BF16)
        m["wo"] = np.ascontiguousarray(Wo[cols].reshape(H_LOCAL, 128, HIDDEN)).astype(BF16)
        in_maps.append(m)
    return in_maps


_NC_CACHE = {}


def kernel(hidden_states, position_ids, Wq, Wk, Wv, Wkc, Wvc, Wqa, Wqg, Wov,
           Wo, _trace=False):
    from concourse.bass_utils import run_bass_kernel_spmd

    B, S, _ = np.asarray(hidden_states).shape
    assert B == 2
    in_maps = _host_inputs(hidden_states, position_ids, Wq, Wk, Wv, Wkc, Wvc,
                           Wqa, Wqg, Wov, Wo, S)
    if S not in _NC_CACHE:
        _NC_CACHE[S] = _build_nc(S)
    nc = _NC_CACHE[S]
    res = run_bass_kernel_spmd(nc, in_maps, list(range(N_CORES)), trace=_trace)
    out = np.zeros((B * S, HIDDEN), dtype=np.float32)
    for r in res.results:
        out += r["out"]
    kernel.last_results = res
    return out.reshape(B, S, HIDDEN)



# revision 5
# speedup vs baseline: 1.0633x; 1.0633x over previous
"""DeepSeek-MLA attention block on 8 Trainium2 NeuronCores.

Sharding: tensor-parallel over heads (16 heads / 8 cores = 2 heads per core).
All per-head projections (k/v compressor, q_a, q_gate, o_v) are head-local;
each core computes a partial output through its slice of Wo and the host sums
the 8 bf16 partials in fp32.

Device layout notes:
  - Activations live transposed: [d, token] tiles so the d_model contraction
    sits on the partition dim.
  - RoPE is folded into the compression matmuls: for a per-token rotation
    x' = c .* x + s .* (P x) (P the fixed rotate-half permutation), any
    projection W satisfies  x'^T W = (c .* x)^T W + (s .* x)^T W_rot  with
    W_rot[e,:] = W[e+32,:] (e<32), -W[e-32,:] (32<=e<64), 0 (e>=64).
  - v has no RoPE, so Wv @ Wvc is folded host-side into a single 2048->64
    per-head projection computed inside the big QKV matmul (640 cols instead
    of 768).  The folded result comes out rank-major; 4 PE-transposes per
    chunk flip it to the keys-major layout the attention matmul needs.
  - Softmax runs unnormalized: exp(scores/8) with the denominator obtained by
    prepending a ones-column to v_c (row 0 of the out_c accumulation is then
    sum_k exp; a zero row in the padded Wov cancels it in the uplift).
  - silu is computed via tanh (same ACT table set as exp, avoiding 1.3us
    activation-table reloads when phase A and B interleave):
    2*silu(x) = x*(1+tanh(x/2)); the 0.5 is folded into Wov host-side.
  - Each kt step's two heads' scores go into one [128,2,512] 2-bank psum
    tile so a single ACT instruction computes exp for both heads (1024 wide).
  - The two batches' phases are software-pipelined: A(b0); then 4 units of
    [B(b0,qc_i) kt-loop with A(b1,chunk_i) matmul-chain steps woven into the
    ACT-bound kt slots]; then B(b1) with the previous qc's Wo groups woven in.
  - PSUM (8 banks): pA,cA = phase-A chains (+ deferred Wo in the tail),
    wide0,wide1 = scores/exp (2 banks each, also uplift + middle Wo slices),
    oc0,oc1 = out_c accumulators.
"""

import collections
import numpy as np
import ml_dtypes

HIDDEN = 2048
N_HEADS = 16
HEAD_DIM = 128
ROPE_DIM = 64
RANK = 64
BASE = 10000.0
N_CORES = 8
H_LOCAL = 2  # heads per core

BF16 = ml_dtypes.bfloat16


def _build_nc(S: int, CH: int = 512):
    """Build the SPMD Bass program for one core (B=2 fixed, seq len S)."""
    import concourse.bacc as bacc
    import concourse.tile as tile
    from concourse import mybir
    from contextlib import ExitStack

    f32 = mybir.dt.float32
    bf16 = mybir.dt.bfloat16
    AF = mybir.ActivationFunctionType
    ALU = mybir.AluOpType

    B = 2
    T = B * S
    PT = HIDDEN // 128          # 16 d_model partition tiles
    KT = S // 128               # 16 key tiles per batch
    NCH = S // CH               # 4 token chunks per batch (phase A)
    QC = S // CH                # 4 q chunks per batch (phase B)
    R1 = RANK + 1               # out_c rows incl. denominator row 0
    NJ = 5                      # qkv col groups: q0,q1,k0,k1,vfold

    nc = bacc.Bacc("TRN2", target_bir_lowering=False, debug=False)

    ht = nc.dram_tensor("ht", [HIDDEN, T], bf16, kind="ExternalInput")
    wqkv = nc.dram_tensor("wqkv", [PT, 128, NJ * 128], bf16, kind="ExternalInput")
    wo_d = nc.dram_tensor("wo", [H_LOCAL, 128, HIDDEN], bf16, kind="ExternalInput")
    wkc_d = nc.dram_tensor("wkc", [128, RANK], bf16, kind="ExternalInput")
    wkcr_d = nc.dram_tensor("wkcr", [128, RANK], bf16, kind="ExternalInput")
    wqa_d = nc.dram_tensor("wqa", [128, RANK], bf16, kind="ExternalInput")
    wqar_d = nc.dram_tensor("wqar", [128, RANK], bf16, kind="ExternalInput")
    wqg_d = nc.dram_tensor("wqg", [128, 128], bf16, kind="ExternalInput")
    wqgr_d = nc.dram_tensor("wqgr", [128, 128], bf16, kind="ExternalInput")
    wovp_d = nc.dram_tensor("wovp", [R1, 128], bf16, kind="ExternalInput")
    ident_d = nc.dram_tensor("ident", [128, 128], bf16, kind="ExternalInput")
    cext_d = nc.dram_tensor("cext", [128, T], bf16, kind="ExternalInput")
    sext_d = nc.dram_tensor("sext", [128, T], bf16, kind="ExternalInput")
    out_d = nc.dram_tensor("out", [T, HIDDEN], bf16, kind="ExternalOutput")

    with tile.TileContext(nc) as tc, ExitStack() as ctx:
        const = ctx.enter_context(tc.tile_pool(name="const", bufs=1))
        hpool = ctx.enter_context(tc.tile_pool(name="hp", bufs=2))
        qkvpool = ctx.enter_context(tc.tile_pool(name="qkvp", bufs=2))
        xpool = ctx.enter_context(tc.tile_pool(name="xp", bufs=1))
        cpool = ctx.enter_context(tc.tile_pool(name="cp", bufs=1))
        epool = ctx.enter_context(tc.tile_pool(name="ep", bufs=1))
        mpool = ctx.enter_context(tc.tile_pool(name="mp", bufs=2))
        opool = ctx.enter_context(tc.tile_pool(name="op", bufs=4))
        psum = ctx.enter_context(tc.tile_pool(name="ps", bufs=1, space="PSUM"))

        # ---- psum bank layout: pin allocation order ----
        # pA(1) cA(1) wide0(2) wide1(2) oc0(1) oc1(1) = 8 banks
        def psA(tag):
            return psum.tile([128, CH], f32, name=tag, tag=tag)

        def ps_wide(tag):
            return psum.tile([128, 2, CH], f32, name=tag, tag=tag)

        def ps_oc(tag):
            return psum.tile([R1, CH], f32, name=tag, tag=tag)

        _pin = [psA("pA"), psA("cA"), ps_wide("wide0"), ps_wide("wide1"),
                ps_oc("oc0"), ps_oc("oc1")]
        del _pin

        # ---- constants ----
        w_qkv = const.tile([128, PT, NJ * 128], bf16, name="wqkv", tag="wqkv")
        for p in range(PT):
            nc.sync.dma_start(out=w_qkv[:, p, :], in_=wqkv[p])
        w_o = const.tile([128, H_LOCAL, HIDDEN], bf16, name="wo", tag="wo")
        for h in range(H_LOCAL):
            nc.sync.dma_start(out=w_o[:, h, :], in_=wo_d[h])
        sm = {}
        for name, d, shp in [
            ("wkc", wkc_d, [128, RANK]), ("wkcr", wkcr_d, [128, RANK]),
            ("wqa", wqa_d, [128, RANK]), ("wqar", wqar_d, [128, RANK]),
            ("wqg", wqg_d, [128, 128]), ("wqgr", wqgr_d, [128, 128]),
            ("wovp", wovp_d, [R1, 128]), ("ident", ident_d, [128, 128]),
        ]:
            t = const.tile(shp, bf16, name=name, tag=name)
            nc.sync.dma_start(out=t[:], in_=d[:])
            sm[name] = t
        cext = const.tile([128, T], bf16, name="cext", tag="cext")
        nc.sync.dma_start(out=cext[:], in_=cext_d[:])
        sext = const.tile([128, T], bf16, name="sext", tag="sext")
        nc.sync.dma_start(out=sext[:], in_=sext_d[:])
        from concourse import library_config
        nc.gpsimd.load_library(library_config.attn)

        # ---- persistent per-batch state ----
        kc_all, qa_all, sg, vc = {}, {}, {}, {}
        for b in range(B):
            kc_all[b] = cpool.tile([128, S], bf16, name=f"kca{b}", tag=f"kca{b}")
            qa_all[b] = cpool.tile([128, S], bf16, name=f"qaa{b}", tag=f"qaa{b}")
            sg[b] = [cpool.tile([128, S], bf16, name=f"sg{b}{h}", tag=f"sg{b}{h}")
                     for h in range(H_LOCAL)]
            vc[b] = [cpool.tile([128, KT, R1], bf16, name=f"vc{b}{h}", tag=f"vc{b}{h}")
                     for h in range(H_LOCAL)]
            for h in range(H_LOCAL):
                nc.vector.memset(vc[b][h][:, :, 0:1], 1.0)
        exps = epool.tile([128, 4, H_LOCAL, CH], bf16, name="exps", tag="exps")

        # ---- filler machinery: each step is ~2 matmuls of PE work ----
        filler = collections.deque()

        def pump(n=1):
            for _ in range(n):
                if not filler:
                    return
                filler.popleft()()

        def flush():
            while filler:
                filler.popleft()()

        # ---------------- Phase A chunk (emitted as steps) ----------------
        def emit_chunkA(b, c, direct):
            base = b * S
            tcol = base + c * CH
            cc = c * CH
            h_t = hpool.tile([128, PT, CH], bf16, name="hch", tag="hch")
            # DMA prefetch issued immediately (engine-independent)
            for p in range(PT):
                nc.sync.dma_start(
                    out=h_t[:, p, :],
                    in_=ht[p * 128:(p + 1) * 128, tcol:tcol + CH],
                )
            csl = cext[:, tcol:tcol + CH]
            ssl = sext[:, tcol:tcol + CH]
            steps = []
            qkv_sb = []
            xq = {}  # rope-multiplied tiles

            def j_chain(j, ps_tag):
                ps = psA(ps_tag)

                def step(k, ps=ps, j=j):
                    def run():
                        for p in (2 * k, 2 * k + 1):
                            nc.tensor.matmul(
                                ps,
                                w_qkv[:, p, j * 128:(j + 1) * 128],
                                h_t[:, p, :],
                                start=(p == 0),
                                stop=(p == PT - 1),
                            )
                        if 2 * k + 1 == PT - 1:
                            sb = qkvpool.tile([128, CH], bf16,
                                              name=f"qsb{j}", tag=f"qsb{j}")
                            nc.vector.tensor_copy(sb, ps)
                            qkv_sb.append(sb)
                            if j == 1:      # q tiles drained -> rope muls
                                for h in range(H_LOCAL):
                                    for nm, sl in (("qc", csl), ("qs", ssl)):
                                        t = xpool.tile(
                                            [128, CH], bf16,
                                            name=f"x{nm}{h}", tag=f"x{nm}{h}")
                                        nc.vector.tensor_mul(t, qkv_sb[h], sl)
                                        xq[f"{nm}{h}"] = t
                            if j == 3:      # k tiles drained
                                for h in range(H_LOCAL):
                                    for nm, sl in (("kc", csl), ("ks", ssl)):
                                        t = xpool.tile(
                                            [128, CH], bf16,
                                            name=f"x{nm}{h}", tag=f"x{nm}{h}")
                                        nc.vector.tensor_mul(t, qkv_sb[2 + h], sl)
                                        xq[f"{nm}{h}"] = t
                    return run
                return [step(k) for k in range(PT // 2)]

            for j in range(NJ):
                steps += j_chain(j, "pA" if j % 2 == 0 else "cA")

            def cmp_kcqa(dst, wc, wr, pref, ps_tag):
                def run():
                    ps = psA(ps_tag)
                    for h in range(H_LOCAL):
                        tp = (0, 64 * h) if h else None
                        nc.tensor.matmul(ps[64 * h:64 * h + 64, :], sm[wc],
                                         xq[f"{pref}c{h}"],
                                         start=True, stop=False, tile_position=tp)
                        nc.tensor.matmul(ps[64 * h:64 * h + 64, :], sm[wr],
                                         xq[f"{pref}s{h}"],
                                         start=False, stop=True, tile_position=tp)
                    nc.vector.tensor_copy(dst[:, cc:cc + CH], ps)
                return run

            steps.append(cmp_kcqa(kc_all[b], "wkc", "wkcr", "k", "cA"))
            steps.append(cmp_kcqa(qa_all[b], "wqa", "wqar", "q", "pA"))

            def cmp_qg(h, ps_tag):
                def run():
                    ps = psA(ps_tag)
                    nc.tensor.matmul(ps, sm["wqg"], xq[f"qc{h}"],
                                     start=True, stop=False)
                    nc.tensor.matmul(ps, sm["wqgr"], xq[f"qs{h}"],
                                     start=False, stop=True)
                    # 2*silu(x) = x * (1 + tanh(x/2)); 0.5 folded into wovp
                    tg = mpool.tile([128, CH], bf16, name="tg", tag="tg")
                    nc.scalar.activation(tg, ps, AF.Tanh, scale=0.5)
                    nc.vector.scalar_tensor_tensor(
                        sg[b][h][:, cc:cc + CH], tg, 1.0, ps,
                        ALU.add, ALU.mult)
                return run

            steps.append(cmp_qg(0, "cA"))
            steps.append(cmp_qg(1, "pA"))

            def vtrans():
                # vfold psum came out rank-major [128=2x64 ranks, CH tokens];
                # PE-transpose 128-token blocks to keys-major for the oc matmul
                ps = psum.tile([128, 4, 128], bf16, name="tV", tag="cA")
                for tt in range(CH // 128):
                    nc.tensor.transpose(
                        ps[:, tt, :],
                        qkv_sb[4][:, tt * 128:(tt + 1) * 128],
                        sm["ident"],
                    )
                for h in range(H_LOCAL):
                    nc.vector.tensor_copy(
                        vc[b][h][:, 4 * c:4 * c + 4, 1:R1],
                        ps[:, :, 64 * h:64 * h + 64],
                    )
            steps.append(vtrans)

            if direct:
                for s in steps:
                    s()
            else:
                filler.extend(steps)

        # ---------------- Phase B q-chunk ----------------
        def emit_qcB(b, qc, defer_wo):
            base = b * S
            qcc = qc * CH
            oc_ps = [ps_oc(f"oc{h}") for h in range(H_LOCAL)]
            wide = [ps_wide("wide0"), ps_wide("wide1")]

            def scores(kt):
                w = wide[kt % 2]
                for h in range(H_LOCAL):
                    nc.tensor.matmul(
                        w[:, h, :],
                        kc_all[b][64 * h:64 * h + 64, kt * 128:(kt + 1) * 128],
                        qa_all[b][64 * h:64 * h + 64, qcc:qcc + CH],
                        start=True, stop=True,
                        tile_position=(64 * h, 0),
                    )
                nc.scalar.activation(
                    exps[:, kt % 4, :, :], w[:, :, :], AF.Exp,
                    scale=float(1.0 / np.sqrt(RANK)))

            def oc(kt):
                for h in range(H_LOCAL):
                    nc.tensor.matmul(
                        oc_ps[h],
                        vc[b][h][:, kt, :],
                        exps[:, kt % 4, h, :],
                        start=(kt == 0),
                        stop=(kt == KT - 1),
                    )

            for kt in range(KT):
                scores(kt)
                if kt > 0:
                    oc(kt - 1)
                pump(1)
            oc(KT - 1)

            gated = []
            for h in range(H_LOCAL):
                den = mpool.tile([1, CH], f32, name=f"den{h}", tag=f"den{h}", bufs=1)
                nc.vector.tensor_copy(den, oc_ps[h][0:1, :])
                oc_sb = mpool.tile([R1, CH], bf16, name=f"ocsb{h}", tag=f"ocsb{h}")
                nc.vector.tensor_copy(oc_sb, oc_ps[h])
                rdet = mpool.tile([1, CH], f32, name=f"rdet{h}", tag=f"rdet{h}", bufs=1)
                nc.vector.reciprocal_approx_fast(rdet, den)
                rdb = mpool.tile([1, CH], bf16, name=f"rdb{h}", tag=f"rdb{h}", bufs=1)
                nc.vector.tensor_copy(rdb, rdet)
                bcast = mpool.tile([128, CH], bf16, name=f"bc{h}", tag=f"bc{h}")
                nc.gpsimd.partition_broadcast(bcast, rdb)
                up = wide[0][:, h, :]
                nc.tensor.matmul(up, sm["wovp"], oc_sb, start=True, stop=True)
                t1 = mpool.tile([128, CH], bf16, name=f"t1{h}", tag=f"t1{h}", bufs=1)
                nc.vector.tensor_mul(t1, up, sg[b][h][:, qcc:qcc + CH])
                g = mpool.tile([128, CH], bf16, name=f"gated{h}", tag=f"gated{h}")
                nc.vector.tensor_mul(g, t1, bcast)
                gated.append(g)

            wo_slices = [wide[0][:, 0, :], wide[0][:, 1, :],
                         wide[1][:, 0, :], wide[1][:, 1, :]]

            def wo_group(gi, ps):
                tt, n = divmod(gi, HIDDEN // CH)

                def run():
                    p = ps() if callable(ps) else ps
                    for h in range(H_LOCAL):
                        nc.tensor.matmul(
                            p,
                            gated[h][:, tt * 128:(tt + 1) * 128],
                            w_o[:, h, n * CH:(n + 1) * CH],
                            start=(h == 0),
                            stop=(h == H_LOCAL - 1),
                        )
                    ost = opool.tile([128, CH], bf16, name="ost", tag="ost")
                    nc.vector.tensor_copy(ost, p)
                    r0 = base + qcc + tt * 128
                    nc.sync.dma_start(
                        out=out_d[r0:r0 + 128, n * CH:(n + 1) * CH], in_=ost)
                return run

            ngroups = (CH // 128) * (HIDDEN // CH)
            if defer_wo:
                # tail: weave Wo groups into the next qc's kt loop on the
                # phase-A banks (free once phase A is done)
                for gi in range(ngroups):
                    tag = "pA" if gi % 2 == 0 else "cA"
                    filler.append(wo_group(gi, (lambda t=tag: psA(t))))
            else:
                for gi in range(ngroups):
                    wo_group(gi, wo_slices[gi % 4])()

        # ---------------- emission schedule ----------------
        for c in range(NCH):
            emit_chunkA(0, c, direct=True)
        for i in range(QC):
            emit_chunkA(1, i, direct=False)
            emit_qcB(0, i, defer_wo=False)
            flush()
        for i in range(QC):
            emit_qcB(1, i, defer_wo=(i < QC - 1))
        flush()

    nc.compile()
    return nc


def _rot_w(w):
    """Fold rotate-half into a projection matrix (see module docstring)."""
    r = np.zeros_like(w)
    r[0:32] = w[32:64]
    r[32:64] = -w[0:32]
    return r


def _host_inputs(hidden_states, position_ids, Wq, Wk, Wv, Wkc, Wvc, Wqa, Wqg,
                 Wov, Wo, S):
    """Build the 8 per-core input maps (all device arrays bf16)."""
    B = 2
    T = B * S
    h = np.asarray(hidden_states, dtype=np.float32).reshape(T, HIDDEN)
    ht = np.ascontiguousarray(h.T).astype(BF16)

    pos = np.asarray(position_ids).reshape(-1).astype(np.float64)
    pos = np.concatenate([pos] * B)  # token order is [b0 tokens, b1 tokens]
    inv_freq = 1.0 / (BASE ** (np.arange(0, ROPE_DIM, 2, dtype=np.float64) / ROPE_DIM))
    freqs = np.outer(pos, inv_freq)                       # [T, 32]
    emb = np.concatenate([freqs, freqs], axis=1)          # [T, 64]
    cext = np.ones((128, T), dtype=np.float32)
    sext = np.zeros((128, T), dtype=np.float32)
    cext[0:ROPE_DIM] = np.cos(emb).T
    sext[0:ROPE_DIM] = np.sin(emb).T

    Wkc = np.asarray(Wkc, np.float32); Wvc = np.asarray(Wvc, np.float32)
    Wqa = np.asarray(Wqa, np.float32); Wqg = np.asarray(Wqg, np.float32)
    Wov = np.asarray(Wov, np.float32)
    # 0.5 factor: kernel computes 2*silu via the tanh identity
    wovp = np.concatenate([np.zeros((1, 128), np.float32), 0.5 * Wov], axis=0)

    shared = {
        "ht": ht,
        "wkc": Wkc.astype(BF16), "wkcr": _rot_w(Wkc).astype(BF16),
        "wqa": Wqa.astype(BF16), "wqar": _rot_w(Wqa).astype(BF16),
        "wqg": Wqg.astype(BF16), "wqgr": _rot_w(Wqg).astype(BF16),
        "wovp": wovp.astype(BF16),
        "ident": np.eye(128, dtype=np.float32).astype(BF16),
        "cext": cext.astype(BF16), "sext": sext.astype(BF16),
    }

    Wq = np.asarray(Wq, np.float32); Wk = np.asarray(Wk, np.float32)
    Wv = np.asarray(Wv, np.float32); Wo = np.asarray(Wo, np.float32)
    in_maps = []
    for c in range(N_CORES):
        cols = slice(c * 256, (c + 1) * 256)
        vf = [Wv[:, c * 256 + 128 * h:c * 256 + 128 * (h + 1)] @ Wvc
              for h in range(H_LOCAL)]                     # each [2048, 64]
        wbig = np.concatenate(
            [Wq[:, cols], Wk[:, cols]] + vf, axis=1)       # [2048, 640]
        m = dict(shared)
        m["wqkv"] = np.ascontiguousarray(
            wbig.reshape(HIDDEN // 128, 128, 640)).astype(BF16)
        m["wo"] = np.ascontiguousarray(
            Wo[cols].reshape(H_LOCAL, 128, HIDDEN)).astype(BF16)
        in_maps.append(m)
    return in_maps


_NC_CACHE = {}


def kernel(hidden_states, position_ids, Wq, Wk, Wv, Wkc, Wvc, Wqa, Wqg, Wov,
           Wo, _trace=False):
    from concourse.bass_utils import run_bass_kernel_spmd

    B, S, _ = np.asarray(hidden_states).shape
    assert B == 2
    in_maps = _host_inputs(hidden_states, position_ids, Wq, Wk, Wv, Wkc, Wvc,
                           Wqa, Wqg, Wov, Wo, S)
    if S not in _NC_CACHE:
        _NC_CACHE[S] = _build_nc(S)
    nc = _NC_CACHE[S]
    res = run_bass_kernel_spmd(nc, in_maps, list(range(N_CORES)), trace=_trace)
    out = np.zeros((B * S, HIDDEN), dtype=np.float32)
    for r in res.results:
        out += np.asarray(r["out"]).astype(np.float32)
    kernel.last_results = res
    return out.reshape(B, S, HIDDEN)


# revision 12
# speedup vs baseline: 1.1398x; 1.0719x over previous
"""DeepSeek-MLA attention block on 8 Trainium2 NeuronCores.

Sharding: tensor-parallel over heads (16 heads / 8 cores = 2 heads per core).
All per-head projections (k/v compressor, q_a, q_gate, o_v) are head-local;
each core computes a partial output through its slice of Wo and the host sums
the 8 bf16 partials in fp32.

Device layout notes:
  - Activations live transposed: [d, token] tiles so the d_model contraction
    sits on the partition dim.
  - RoPE is folded into the compression matmuls: for a per-token rotation
    x' = c .* x + s .* (P x) (P the fixed rotate-half permutation), any
    projection W satisfies  x'^T W = (c .* x)^T W + (s .* x)^T W_rot  with
    W_rot[e,:] = W[e+32,:] (e<32), -W[e-32,:] (32<=e<64), 0 (e>=64).
  - v has no RoPE, so Wv @ Wvc is folded host-side into a single 2048->64
    per-head projection computed inside the big QKV matmul (640 cols instead
    of 768).  The folded result comes out rank-major; 4 PE-transposes per
    chunk flip it to the keys-major layout the attention matmul needs.
  - Softmax runs unnormalized: exp(scores/8) with the denominator obtained by
    prepending a ones-column to v_c (row 0 of the out_c accumulation is then
    sum_k exp; a zero row in the padded Wov cancels it in the uplift).
  - silu is computed via tanh (same ACT table set as exp, avoiding 1.3us
    activation-table reloads when phase A and B interleave):
    2*silu(x) = x*(1+tanh(x/2)); the 0.5 is folded into Wov host-side.
  - Each kt step's two heads' scores go into one [128,2,512] 2-bank psum
    tile so a single ACT instruction computes exp for both heads (1024 wide).
  - The two batches' phases are software-pipelined: A(b0); then 4 units of
    [B(b0,qc_i) kt-loop with A(b1,chunk_i) matmul-chain steps woven into the
    ACT-bound kt slots]; then B(b1) with the previous qc's Wo groups woven in.
  - PSUM (8 banks): pA,cA = phase-A chains (+ deferred Wo in the tail),
    wide0,wide1 = scores/exp (2 banks each, also uplift + middle Wo slices),
    oc0,oc1 = out_c accumulators.
"""

import collections
import numpy as np
import ml_dtypes

HIDDEN = 2048
N_HEADS = 16
HEAD_DIM = 128
ROPE_DIM = 64
RANK = 64
BASE = 10000.0
N_CORES = 8
H_LOCAL = 2  # heads per core

BF16 = ml_dtypes.bfloat16


def _build_nc(S: int, CH: int = 512):
    """Build the SPMD Bass program for one core (B=2 fixed, seq len S)."""
    import concourse.bacc as bacc
    import concourse.tile as tile
    from concourse import mybir
    from contextlib import ExitStack

    f32 = mybir.dt.float32
    bf16 = mybir.dt.bfloat16
    AF = mybir.ActivationFunctionType
    ALU = mybir.AluOpType

    B = 2
    T = B * S
    PT = HIDDEN // 128          # 16 d_model partition tiles
    KT = S // 128               # 16 key tiles per batch
    NCH = S // CH               # 4 token chunks per batch (phase A)
    QC = S // CH                # 4 q chunks per batch (phase B)
    R1 = RANK + 1               # out_c rows incl. denominator row 0
    NJ = 5                      # qkv col groups: q0,q1,k0,k1,vfold

    nc = bacc.Bacc("TRN2", target_bir_lowering=False, debug=False)

    ht = nc.dram_tensor("ht", [HIDDEN, T], bf16, kind="ExternalInput")
    wqkv = nc.dram_tensor("wqkv", [PT, 128, NJ * 128], bf16, kind="ExternalInput")
    wo_d = nc.dram_tensor("wo", [H_LOCAL, 128, HIDDEN], bf16, kind="ExternalInput")
    wkc_d = nc.dram_tensor("wkc", [128, RANK], bf16, kind="ExternalInput")
    wkcr_d = nc.dram_tensor("wkcr", [128, RANK], bf16, kind="ExternalInput")
    wqa_d = nc.dram_tensor("wqa", [128, RANK], bf16, kind="ExternalInput")
    wqar_d = nc.dram_tensor("wqar", [128, RANK], bf16, kind="ExternalInput")
    wqg_d = nc.dram_tensor("wqg", [128, 128], bf16, kind="ExternalInput")
    wqgr_d = nc.dram_tensor("wqgr", [128, 128], bf16, kind="ExternalInput")
    wovp_d = nc.dram_tensor("wovp", [R1, 128], bf16, kind="ExternalInput")
    ident_d = nc.dram_tensor("ident", [128, 128], bf16, kind="ExternalInput")
    cext_d = nc.dram_tensor("cext", [128, T], bf16, kind="ExternalInput")
    sext_d = nc.dram_tensor("sext", [128, T], bf16, kind="ExternalInput")
    out_d = nc.dram_tensor("out", [T, HIDDEN], bf16, kind="ExternalOutput")

    with tile.TileContext(nc) as tc, ExitStack() as ctx:
        const = ctx.enter_context(tc.tile_pool(name="const", bufs=1))
        hpool = ctx.enter_context(tc.tile_pool(name="hp", bufs=2))
        qkvpool = ctx.enter_context(tc.tile_pool(name="qkvp", bufs=2))
        xpool = ctx.enter_context(tc.tile_pool(name="xp", bufs=1))
        cpool = ctx.enter_context(tc.tile_pool(name="cp", bufs=1))
        epool = ctx.enter_context(tc.tile_pool(name="ep", bufs=1))
        mpool = ctx.enter_context(tc.tile_pool(name="mp", bufs=2))
        opool = ctx.enter_context(tc.tile_pool(name="op", bufs=4))
        psum = ctx.enter_context(tc.tile_pool(name="ps", bufs=1, space="PSUM"))

        # ---- psum bank layout: pin allocation order ----
        # pA(1) cA(1) wide0(2) wide1(2) oc0(1) oc1(1) = 8 banks
        def psA(tag):
            return psum.tile([128, CH], f32, name=tag, tag=tag)

        def ps_wide(tag):
            return psum.tile([128, 2, CH], f32, name=tag, tag=tag)

        def ps_oc(tag):
            return psum.tile([R1, CH], f32, name=tag, tag=tag)

        _pin = [psA("pA"), psA("cA"), ps_wide("wide0"), ps_wide("wide1"),
                ps_oc("oc0"), ps_oc("oc1")]
        del _pin

        # ---- constants ----
        # DMA order matters for startup latency: wqkv p-slices first (first
        # j-chain needs p=0 almost immediately), small weights next, cext /
        # sext after the first hidden chunk, w_o (needed only ~80us in) last.
        w_qkv = const.tile([128, PT, NJ * 128], bf16, name="wqkv", tag="wqkv")
        sm = {}
        for name, d, shp in [
            ("wkc", wkc_d, [128, RANK]), ("wkcr", wkcr_d, [128, RANK]),
            ("wqa", wqa_d, [128, RANK]), ("wqar", wqar_d, [128, RANK]),
            ("wqg", wqg_d, [128, 128]), ("wqgr", wqgr_d, [128, 128]),
            ("wovp", wovp_d, [R1, 128]), ("ident", ident_d, [128, 128]),
        ]:
            t = const.tile(shp, bf16, name=name, tag=name)
            nc.sync.dma_start(out=t[:], in_=d[:])
            sm[name] = t
        cext = const.tile([128, T], bf16, name="cext", tag="cext")
        sext = const.tile([128, T], bf16, name="sext", tag="sext")
        w_o = const.tile([128, H_LOCAL, HIDDEN], bf16, name="wo", tag="wo")
        from concourse import library_config
        nc.gpsimd.load_library(library_config.attn)

        # ---- persistent per-batch state ----
        kc_all, qa_all, sg, vc = {}, {}, {}, {}
        for b in range(B):
            kc_all[b] = cpool.tile([128, S], bf16, name=f"kca{b}", tag=f"kca{b}")
            qa_all[b] = cpool.tile([128, S], bf16, name=f"qaa{b}", tag=f"qaa{b}")
            sg[b] = [cpool.tile([128, S], bf16, name=f"sg{b}{h}", tag=f"sg{b}{h}")
                     for h in range(H_LOCAL)]
            vc[b] = [cpool.tile([128, KT, R1], bf16, name=f"vc{b}{h}", tag=f"vc{b}{h}")
                     for h in range(H_LOCAL)]
            for h in range(H_LOCAL):
                nc.vector.memset(vc[b][h][:, :, 0:1], 1.0)
        NSLOT = 8   # exp lookahead slots: lets ACT run ahead of the oc chain
        exps = epool.tile([128, NSLOT, H_LOCAL, CH], bf16, name="exps", tag="exps")

        # ---- filler machinery: each step is ~2 matmuls of PE work ----
        filler = collections.deque()

        def pump(n=1):
            for _ in range(n):
                if not filler:
                    return
                filler.popleft()()

        def flush():
            while filler:
                filler.popleft()()

        # ---------------- Phase A chunk (emitted as steps) ----------------
        def chunk_dma(b, c):
            tcol = b * S + c * CH
            h_t = hpool.tile([128, PT, CH], bf16, name="hch", tag="hch")
            for p in range(PT):
                nc.sync.dma_start(
                    out=h_t[:, p, :],
                    in_=ht[p * 128:(p + 1) * 128, tcol:tcol + CH],
                )
            return h_t

        def emit_chunkA(b, c, direct, h_t=None):
            base = b * S
            tcol = base + c * CH
            cc = c * CH
            if h_t is None:
                h_t = chunk_dma(b, c)
            csl = cext[:, tcol:tcol + CH]
            ssl = sext[:, tcol:tcol + CH]
            steps = []
            qkv_sb = []
            xq = {}  # rope-multiplied tiles

            def j_chain(j, ps_tag):
                ps = psA(ps_tag)

                def step(k, ps=ps, j=j):
                    def run():
                        for p in (2 * k, 2 * k + 1):
                            nc.tensor.matmul(
                                ps,
                                w_qkv[:, p, j * 128:(j + 1) * 128],
                                h_t[:, p, :],
                                start=(p == 0),
                                stop=(p == PT - 1),
                            )
                        if 2 * k + 1 == PT - 1:
                            sb = qkvpool.tile([128, CH], bf16,
                                              name=f"qsb{j}", tag=f"qsb{j}")
                            nc.vector.tensor_copy(sb, ps)
                            qkv_sb.append(sb)
                            if j == 1:      # q tiles drained -> rope muls
                                for h in range(H_LOCAL):
                                    for nm, sl in (("qc", csl), ("qs", ssl)):
                                        t = xpool.tile(
                                            [128, CH], bf16,
                                            name=f"x{nm}{h}", tag=f"x{nm}{h}")
                                        nc.vector.tensor_mul(t, qkv_sb[h], sl)
                                        xq[f"{nm}{h}"] = t
                            if j == 3:      # k tiles drained
                                for h in range(H_LOCAL):
                                    for nm, sl in (("kc", csl), ("ks", ssl)):
                                        t = xpool.tile(
                                            [128, CH], bf16,
                                            name=f"x{nm}{h}", tag=f"x{nm}{h}")
                                        nc.vector.tensor_mul(t, qkv_sb[2 + h], sl)
                                        xq[f"{nm}{h}"] = t
                    return run
                return [step(k) for k in range(PT // 2)]

            for j in range(NJ):
                steps += j_chain(j, "pA" if j % 2 == 0 else "cA")

            def cmp_kcqa(dst, wc, wr, pref, ps_tag):
                def run():
                    ps = psA(ps_tag)
                    for h in range(H_LOCAL):
                        tp = (0, 64 * h) if h else None
                        nc.tensor.matmul(ps[64 * h:64 * h + 64, :], sm[wc],
                                         xq[f"{pref}c{h}"],
                                         start=True, stop=False, tile_position=tp)
                        nc.tensor.matmul(ps[64 * h:64 * h + 64, :], sm[wr],
                                         xq[f"{pref}s{h}"],
                                         start=False, stop=True, tile_position=tp)
                    nc.vector.tensor_copy(dst[:, cc:cc + CH], ps)
                return run

            steps.append(cmp_kcqa(kc_all[b], "wkc", "wkcr", "k", "cA"))
            steps.append(cmp_kcqa(qa_all[b], "wqa", "wqar", "q", "pA"))

            def cmp_qg(h, ps_tag):
                def run():
                    ps = psA(ps_tag)
                    nc.tensor.matmul(ps, sm["wqg"], xq[f"qc{h}"],
                                     start=True, stop=False)
                    nc.tensor.matmul(ps, sm["wqgr"], xq[f"qs{h}"],
                                     start=False, stop=True)
                    # 2*silu(x) = x * (1 + tanh(x/2)); 0.5 folded into wovp
                    tg = mpool.tile([128, CH], bf16, name="tg", tag="tg")
                    nc.scalar.activation(tg, ps, AF.Tanh, scale=0.5)
                    nc.vector.scalar_tensor_tensor(
                        sg[b][h][:, cc:cc + CH], tg, 1.0, ps,
                        ALU.add, ALU.mult)
                return run

            steps.append(cmp_qg(0, "cA"))
            steps.append(cmp_qg(1, "pA"))

            def vtrans():
                # vfold psum came out rank-major [128=2x64 ranks, CH tokens];
                # PE-transpose 128-token blocks to keys-major for the oc matmul
                ps = psum.tile([128, 4, 128], bf16, name="tV", tag="cA")
                for tt in range(CH // 128):
                    nc.tensor.transpose(
                        ps[:, tt, :],
                        qkv_sb[4][:, tt * 128:(tt + 1) * 128],
                        sm["ident"],
                    )
                for h in range(H_LOCAL):
                    nc.vector.tensor_copy(
                        vc[b][h][:, 4 * c:4 * c + 4, 1:R1],
                        ps[:, :, 64 * h:64 * h + 64],
                    )
            steps.append(vtrans)

            if direct:
                for s in steps:
                    s()
            else:
                filler.extend(steps)

        # ---------------- Phase B q-chunk ----------------
        def emit_qcB(b, qc, oc_tags, wo_on_ocpair):
            base = b * S
            qcc = qc * CH
            oc_ps = [psum.tile([R1, CH], f32, name=t, tag=t) for t in oc_tags]
            wide = [ps_wide("wide0"), ps_wide("wide1")]

            def scores(kt):
                w = wide[kt % 2]
                for h in range(H_LOCAL):
                    nc.tensor.matmul(
                        w[:, h, :],
                        kc_all[b][64 * h:64 * h + 64, kt * 128:(kt + 1) * 128],
                        qa_all[b][64 * h:64 * h + 64, qcc:qcc + CH],
                        start=True, stop=True,
                        tile_position=(64 * h, 0),
                    )
                nc.scalar.activation(
                    exps[:, kt % NSLOT, :, :], w[:, :, :], AF.Exp,
                    scale=float(1.0 / np.sqrt(RANK)))

            def oc(kt):
                for h in range(H_LOCAL):
                    nc.tensor.matmul(
                        oc_ps[h],
                        vc[b][h][:, kt, :],
                        exps[:, kt % NSLOT, h, :],
                        start=(kt == 0),
                        stop=(kt == KT - 1),
                    )

            for kt in range(KT):
                scores(kt)
                if kt > 0:
                    oc(kt - 1)
                pump(1)
            oc(KT - 1)

            gated = []
            for h in range(H_LOCAL):
                # den/recip/broadcast chain first (longest serial path)
                den = mpool.tile([1, CH], f32, name=f"den{h}", tag=f"den{h}", bufs=1)
                nc.vector.tensor_copy(den, oc_ps[h][0:1, :])
                rdet = mpool.tile([1, CH], f32, name=f"rdet{h}", tag=f"rdet{h}", bufs=1)
                nc.vector.reciprocal_approx_fast(rdet, den)
                rdb = mpool.tile([1, CH], bf16, name=f"rdb{h}", tag=f"rdb{h}", bufs=1)
                nc.vector.tensor_copy(rdb, rdet)
                bcast = mpool.tile([128, CH], bf16, name=f"bc{h}", tag=f"bc{h}")
                nc.gpsimd.partition_broadcast(bcast, rdb)
                oc_sb = mpool.tile([R1, CH], bf16, name=f"ocsb{h}", tag=f"ocsb{h}")
                nc.vector.tensor_copy(oc_sb, oc_ps[h])
                if wo_on_ocpair:
                    # tail: uplift on this qc's freed oc bank so wide frees
                    # right after the last exp -> next qc's scores start early
                    up = psum.tile([128, CH], f32, name=f"up{h}",
                                   tag=oc_tags[h])
                else:
                    up = wide[0][:, h, :]
                nc.tensor.matmul(up, sm["wovp"], oc_sb, start=True, stop=True)
                t1 = mpool.tile([128, CH], bf16, name=f"t1{h}", tag=f"t1{h}", bufs=1)
                nc.vector.tensor_mul(t1, up, sg[b][h][:, qcc:qcc + CH])
                g = mpool.tile([128, CH], bf16, name=f"gated{h}", tag=f"gated{h}")
                nc.vector.tensor_mul(g, t1, bcast)
                gated.append(g)

            wo_slices = (
                # tail: Wo on this qc's freed oc banks so the next qc's kt
                # loop (wide + other oc pair) can run fully concurrent
                [(lambda t=t: psum.tile([128, CH], f32, name=f"wo{t}", tag=t))
                 for t in oc_tags]
                if wo_on_ocpair else
                [wide[0][:, 0, :], wide[0][:, 1, :],
                 wide[1][:, 0, :], wide[1][:, 1, :]]
            )

            ngroups = (CH // 128) * (HIDDEN // CH)
            for gi in range(ngroups):
                ps = wo_slices[gi % len(wo_slices)]
                p = ps() if callable(ps) else ps
                for h in range(H_LOCAL):
                    nc.tensor.matmul(
                        p,
                        gated[h][:, (gi // (HIDDEN // CH)) * 128:
                                 (gi // (HIDDEN // CH)) * 128 + 128],
                        w_o[:, h, (gi % (HIDDEN // CH)) * CH:
                            (gi % (HIDDEN // CH) + 1) * CH],
                        start=(h == 0),
                        stop=(h == H_LOCAL - 1),
                    )
                ost = opool.tile([128, CH], bf16, name="ost", tag="ost")
                nc.vector.tensor_copy(ost, p)
                r0 = base + qcc + (gi // (HIDDEN // CH)) * 128
                nc.sync.dma_start(
                    out=out_d[r0:r0 + 128,
                              (gi % (HIDDEN // CH)) * CH:
                              (gi % (HIDDEN // CH) + 1) * CH],
                    in_=ost)

        # ---------------- emission schedule ----------------
        # startup: interleave wqkv p-slices with chunk 0's hidden p-slices so
        # the first j-chain matmul can start after ~0.3MB of DMA, not 5MB
        h_t0 = hpool.tile([128, PT, CH], bf16, name="hch", tag="hch")
        for p in range(PT):
            nc.sync.dma_start(out=w_qkv[:, p, :], in_=wqkv[p])
            nc.sync.dma_start(out=h_t0[:, p, :],
                              in_=ht[p * 128:(p + 1) * 128, 0:CH])
        nc.sync.dma_start(out=cext[:], in_=cext_d[:])
        nc.sync.dma_start(out=sext[:], in_=sext_d[:])
        emit_chunkA(0, 0, direct=True, h_t=h_t0)
        for c in range(1, NCH):
            emit_chunkA(0, c, direct=True)
        for h in range(H_LOCAL):
            nc.sync.dma_start(out=w_o[:, h, :], in_=wo_d[h])
        for i in range(QC):
            emit_chunkA(1, i, direct=False)
            emit_qcB(0, i, ("oc0", "oc1"), wo_on_ocpair=False)
            flush()
        for i in range(QC):
            tags = ("oc0", "oc1") if i % 2 == 0 else ("pA", "cA")
            emit_qcB(1, i, tags, wo_on_ocpair=True)
        flush()

    nc.compile()
    return nc


def _rot_w(w):
    """Fold rotate-half into a projection matrix (see module docstring)."""
    r = np.zeros_like(w)
    r[0:32] = w[32:64]
    r[32:64] = -w[0:32]
    return r


def _host_inputs(hidden_states, position_ids, Wq, Wk, Wv, Wkc, Wvc, Wqa, Wqg,
                 Wov, Wo, S):
    """Build the 8 per-core input maps (all device arrays bf16)."""
    B = 2
    T = B * S
    h = np.asarray(hidden_states, dtype=np.float32).reshape(T, HIDDEN)
    ht = np.ascontiguousarray(h.T).astype(BF16)

    pos = np.asarray(position_ids).reshape(-1).astype(np.float64)
    pos = np.concatenate([pos] * B)  # token order is [b0 tokens, b1 tokens]
    inv_freq = 1.0 / (BASE ** (np.arange(0, ROPE_DIM, 2, dtype=np.float64) / ROPE_DIM))
    freqs = np.outer(pos, inv_freq)                       # [T, 32]
    emb = np.concatenate([freqs, freqs], axis=1)          # [T, 64]
    cext = np.ones((128, T), dtype=np.float32)
    sext = np.zeros((128, T), dtype=np.float32)
    cext[0:ROPE_DIM] = np.cos(emb).T
    sext[0:ROPE_DIM] = np.sin(emb).T

    Wkc = np.asarray(Wkc, np.float32); Wvc = np.asarray(Wvc, np.float32)
    Wqa = np.asarray(Wqa, np.float32); Wqg = np.asarray(Wqg, np.float32)
    Wov = np.asarray(Wov, np.float32)
    # 0.5 factor: kernel computes 2*silu via the tanh identity
    wovp = np.concatenate([np.zeros((1, 128), np.float32), 0.5 * Wov], axis=0)

    shared = {
        "ht": ht,
        "wkc": Wkc.astype(BF16), "wkcr": _rot_w(Wkc).astype(BF16),
        "wqa": Wqa.astype(BF16), "wqar": _rot_w(Wqa).astype(BF16),
        "wqg": Wqg.astype(BF16), "wqgr": _rot_w(Wqg).astype(BF16),
        "wovp": wovp.astype(BF16),
        "ident": np.eye(128, dtype=np.float32).astype(BF16),
        "cext": cext.astype(BF16), "sext": sext.astype(BF16),
    }

    Wq = np.asarray(Wq, np.float32); Wk = np.asarray(Wk, np.float32)
    Wv = np.asarray(Wv, np.float32); Wo = np.asarray(Wo, np.float32)
    in_maps = []
    for c in range(N_CORES):
        cols = slice(c * 256, (c + 1) * 256)
        vf = [Wv[:, c * 256 + 128 * h:c * 256 + 128 * (h + 1)] @ Wvc
              for h in range(H_LOCAL)]                     # each [2048, 64]
        wbig = np.concatenate(
            [Wq[:, cols], Wk[:, cols]] + vf, axis=1)       # [2048, 640]
        m = dict(shared)
        m["wqkv"] = np.ascontiguousarray(
            wbig.reshape(HIDDEN // 128, 128, 640)).astype(BF16)
        m["wo"] = np.ascontiguousarray(
            Wo[cols].reshape(H_LOCAL, 128, HIDDEN)).astype(BF16)
        in_maps.append(m)
    return in_maps


_NC_CACHE = {}


def kernel(hidden_states, position_ids, Wq, Wk, Wv, Wkc, Wvc, Wqa, Wqg, Wov,
           Wo, _trace=False):
    from concourse.bass_utils import run_bass_kernel_spmd

    B, S, _ = np.asarray(hidden_states).shape
    assert B == 2
    in_maps = _host_inputs(hidden_states, position_ids, Wq, Wk, Wv, Wkc, Wvc,
                           Wqa, Wqg, Wov, Wo, S)
    if S not in _NC_CACHE:
        _NC_CACHE[S] = _build_nc(S)
    nc = _NC_CACHE[S]
    res = run_bass_kernel_spmd(nc, in_maps, list(range(N_CORES)), trace=_trace)
    out = np.zeros((B * S, HIDDEN), dtype=np.float32)
    for r in res.results:
        out += np.asarray(r["out"]).astype(np.float32)
    kernel.last_results = res
    return out.reshape(B, S, HIDDEN)


# revision 22
# speedup vs baseline: 1.1912x; 1.0451x over previous
"""DeepSeek-MLA attention block on 8 Trainium2 NeuronCores.

Sharding: tensor-parallel over heads (16 heads / 8 cores = 2 heads per core).
All per-head projections (k/v compressor, q_a, q_gate, o_v) are head-local;
each core computes a partial output through its slice of Wo and the host sums
the 8 bf16 partials in fp32.

Device layout notes:
  - Activations live transposed: [d, token] tiles so the d_model contraction
    sits on the partition dim.
  - RoPE is folded into the compression matmuls: for a per-token rotation
    x' = c .* x + s .* (P x) (P the fixed rotate-half permutation), any
    projection W satisfies  x'^T W = (c .* x)^T W + (s .* x)^T W_rot  with
    W_rot[e,:] = W[e+32,:] (e<32), -W[e-32,:] (32<=e<64), 0 (e>=64).
  - v has no RoPE, so Wv @ Wvc is folded host-side into a single 2048->64
    per-head projection computed inside the big QKV matmul (640 cols instead
    of 768).  The folded result comes out rank-major; 4 PE-transposes per
    chunk flip it to the keys-major layout the attention matmul needs.
  - Softmax runs unnormalized: exp(scores/8) with the denominator obtained by
    prepending a ones-column to v_c (row 0 of the out_c accumulation is then
    sum_k exp; a zero row in the padded Wov cancels it in the uplift).
  - silu is computed via tanh (same ACT table set as exp, avoiding 1.3us
    activation-table reloads when phase A and B interleave):
    2*silu(x) = x*(1+tanh(x/2)); the 0.5 is folded into Wov host-side.
  - Each kt step's two heads' scores go into one [128,2,512] 2-bank psum
    tile so a single ACT instruction computes exp for both heads (1024 wide).
  - The two batches' phases are software-pipelined: A(b0); then 4 units of
    [B(b0,qc_i) kt-loop with A(b1,chunk_i) matmul-chain steps woven into the
    ACT-bound kt slots]; then B(b1) with the previous qc's Wo groups woven in.
  - PSUM (8 banks): pA,cA = phase-A chains (+ deferred Wo in the tail),
    wide0,wide1 = scores/exp (2 banks each, also uplift + middle Wo slices),
    oc0,oc1 = out_c accumulators.
"""

import collections
import numpy as np
import ml_dtypes

HIDDEN = 2048
N_HEADS = 16
HEAD_DIM = 128
ROPE_DIM = 64
RANK = 64
BASE = 10000.0
N_CORES = 8
H_LOCAL = 2  # heads per core

BF16 = ml_dtypes.bfloat16


def _build_nc(S: int, CH: int = 512):
    """Build the SPMD Bass program for one core (B=2 fixed, seq len S)."""
    import concourse.bacc as bacc
    import concourse.tile as tile
    from concourse import mybir
    from contextlib import ExitStack

    f32 = mybir.dt.float32
    bf16 = mybir.dt.bfloat16
    AF = mybir.ActivationFunctionType
    ALU = mybir.AluOpType

    B = 2
    T = B * S
    PT = HIDDEN // 128          # 16 d_model partition tiles
    KT = S // 128               # 16 key tiles per batch
    NCH = S // CH               # 4 token chunks per batch (phase A)
    QC = S // CH                # 4 q chunks per batch (phase B)
    R1 = RANK + 1               # out_c rows incl. denominator row 0
    NJ = 5                      # qkv col groups: q0,q1,k0,k1,vfold

    nc = bacc.Bacc("TRN2", target_bir_lowering=False, debug=False)

    ht = nc.dram_tensor("ht", [HIDDEN, T], bf16, kind="ExternalInput")
    wqkv = nc.dram_tensor("wqkv", [PT, 128, NJ * 128], bf16, kind="ExternalInput")
    wo_d = nc.dram_tensor("wo", [H_LOCAL, 128, HIDDEN], bf16, kind="ExternalInput")
    wkc_d = nc.dram_tensor("wkc", [128, RANK], bf16, kind="ExternalInput")
    wkcr_d = nc.dram_tensor("wkcr", [128, RANK], bf16, kind="ExternalInput")
    wqa_d = nc.dram_tensor("wqa", [128, RANK], bf16, kind="ExternalInput")
    wqar_d = nc.dram_tensor("wqar", [128, RANK], bf16, kind="ExternalInput")
    wqg_d = nc.dram_tensor("wqg", [128, 128], bf16, kind="ExternalInput")
    wqgr_d = nc.dram_tensor("wqgr", [128, 128], bf16, kind="ExternalInput")
    wovp_d = nc.dram_tensor("wovp", [R1, 128], bf16, kind="ExternalInput")
    ident_d = nc.dram_tensor("ident", [128, 128], bf16, kind="ExternalInput")
    # only the 64 rope rows carry data; rows 64-127 are memset on device
    cext_d = nc.dram_tensor("cext", [ROPE_DIM, T], bf16, kind="ExternalInput")
    sext_d = nc.dram_tensor("sext", [ROPE_DIM, T], bf16, kind="ExternalInput")
    out_d = nc.dram_tensor("out", [T, HIDDEN], bf16, kind="ExternalOutput")

    with tile.TileContext(nc) as tc, ExitStack() as ctx:
        const = ctx.enter_context(tc.tile_pool(name="const", bufs=1))
        hpool = ctx.enter_context(tc.tile_pool(name="hp", bufs=2))
        qkvpool = ctx.enter_context(tc.tile_pool(name="qkvp", bufs=2))
        xpool = ctx.enter_context(tc.tile_pool(name="xp", bufs=1))
        cpool = ctx.enter_context(tc.tile_pool(name="cp", bufs=1))
        epool = ctx.enter_context(tc.tile_pool(name="ep", bufs=1))
        mpool = ctx.enter_context(tc.tile_pool(name="mp", bufs=2))
        opool = ctx.enter_context(tc.tile_pool(name="op", bufs=4))
        psum = ctx.enter_context(tc.tile_pool(name="ps", bufs=1, space="PSUM"))

        # ---- psum bank layout: pin allocation order ----
        # pA(1) cA(1) wide0(2) wide1(2) oc0(1) oc1(1) = 8 banks
        def psA(tag):
            return psum.tile([128, CH], f32, name=tag, tag=tag)

        def ps_wide(tag):
            return psum.tile([128, 2, CH], f32, name=tag, tag=tag)

        def ps_oc(tag):
            return psum.tile([R1, CH], f32, name=tag, tag=tag)

        _pin = [psA("pA"), psA("cA"), ps_wide("wide0"), ps_wide("wide1"),
                ps_oc("oc0"), ps_oc("oc1")]
        del _pin

        # ---- constants ----
        # DMA order matters for startup latency: wqkv p-slices first (first
        # j-chain needs p=0 almost immediately), small weights next, cext /
        # sext after the first hidden chunk, w_o (needed only ~80us in) last.
        w_qkv = const.tile([128, PT, NJ * 128], bf16, name="wqkv", tag="wqkv")
        sm = {}
        for name, d, shp in [
            ("wkc", wkc_d, [128, RANK]), ("wkcr", wkcr_d, [128, RANK]),
            ("wqa", wqa_d, [128, RANK]), ("wqar", wqar_d, [128, RANK]),
            ("wqg", wqg_d, [128, 128]), ("wqgr", wqgr_d, [128, 128]),
            ("wovp", wovp_d, [R1, 128]), ("ident", ident_d, [128, 128]),
        ]:
            t = const.tile(shp, bf16, name=name, tag=name)
            nc.sync.dma_start(out=t[:], in_=d[:])
            sm[name] = t
        cext = const.tile([128, T], bf16, name="cext", tag="cext")
        sext = const.tile([128, T], bf16, name="sext", tag="sext")
        w_o = const.tile([128, H_LOCAL, HIDDEN], bf16, name="wo", tag="wo")
        from concourse import library_config
        nc.gpsimd.load_library(library_config.attn)

        # ---- persistent per-batch state ----
        kc_all, qa_all, sg, vc = {}, {}, {}, {}
        for b in range(B):
            kc_all[b] = cpool.tile([128, S], bf16, name=f"kca{b}", tag=f"kca{b}")
            qa_all[b] = cpool.tile([128, S], bf16, name=f"qaa{b}", tag=f"qaa{b}")
            sg[b] = [cpool.tile([128, S], bf16, name=f"sg{b}{h}", tag=f"sg{b}{h}")
                     for h in range(H_LOCAL)]
            vc[b] = [cpool.tile([128, KT, R1], bf16, name=f"vc{b}{h}", tag=f"vc{b}{h}")
                     for h in range(H_LOCAL)]
            for h in range(H_LOCAL):
                nc.vector.memset(vc[b][h][:, :, 0:1], 1.0)
        NSLOT = 8   # exp lookahead slots: lets ACT run ahead of the oc chain
        exps = epool.tile([128, NSLOT, H_LOCAL, CH], bf16, name="exps", tag="exps")

        # ---- filler machinery: each step is ~2 matmuls of PE work ----
        filler = collections.deque()

        def pump(n=1):
            for _ in range(n):
                if not filler:
                    return
                filler.popleft()()

        def flush():
            while filler:
                filler.popleft()()

        # ---------------- Phase A chunk (emitted as steps) ----------------
        def chunk_dma(b, c):
            tcol = b * S + c * CH
            h_t = hpool.tile([128, PT, CH], bf16, name="hch", tag="hch")
            for p in range(PT):
                nc.sync.dma_start(
                    out=h_t[:, p, :],
                    in_=ht[p * 128:(p + 1) * 128, tcol:tcol + CH],
                )
            return h_t

        def emit_chunkA(b, c, direct, h_t=None):
            base = b * S
            tcol = base + c * CH
            cc = c * CH
            if h_t is None:
                h_t = chunk_dma(b, c)
            # direct (front) chunks ping-pong 2 banks for max PE rate;
            # filler (middle) chunks stay on pA only, leaving cA for the
            # concurrent Wo groups of the interleaved B q-chunks
            bankj = (lambda j: "pA" if j % 2 == 0 else "cA") if direct \
                else (lambda j: "pA")
            csl = cext[:, tcol:tcol + CH]
            ssl = sext[:, tcol:tcol + CH]
            steps = []
            qkv_sb = []
            xq = {}  # rope-multiplied tiles

            def j_chain(j, ps_tag):
                ps = psA(ps_tag)

                def step(k, ps=ps, j=j):
                    def run():
                        for p in (2 * k, 2 * k + 1):
                            nc.tensor.matmul(
                                ps,
                                w_qkv[:, p, j * 128:(j + 1) * 128],
                                h_t[:, p, :],
                                start=(p == 0),
                                stop=(p == PT - 1),
                            )
                        if 2 * k + 1 == PT - 1:
                            sb = qkvpool.tile([128, CH], bf16,
                                              name=f"qsb{j}", tag=f"qsb{j}")
                            nc.vector.tensor_copy(sb, ps)
                            qkv_sb.append(sb)
                            if j == 1:      # q tiles drained -> rope muls
                                for h in range(H_LOCAL):
                                    for nm, sl in (("qc", csl), ("qs", ssl)):
                                        t = xpool.tile(
                                            [128, CH], bf16,
                                            name=f"x{nm}{h}", tag=f"x{nm}{h}")
                                        nc.vector.tensor_mul(t, qkv_sb[h], sl)
                                        xq[f"{nm}{h}"] = t
                            if j == 3:      # k tiles drained
                                for h in range(H_LOCAL):
                                    for nm, sl in (("kc", csl), ("ks", ssl)):
                                        t = xpool.tile(
                                            [128, CH], bf16,
                                            name=f"x{nm}{h}", tag=f"x{nm}{h}")
                                        nc.vector.tensor_mul(t, qkv_sb[2 + h], sl)
                                        xq[f"{nm}{h}"] = t
                    return run
                return [step(k) for k in range(PT // 2)]

            for j in range(NJ):
                steps += j_chain(j, bankj(j))

            def cmp_kcqa(dst, wc, wr, pref, ps_tag):
                def run():
                    ps = psA(ps_tag)
                    for h in range(H_LOCAL):
                        tp = (0, 64 * h) if h else None
                        nc.tensor.matmul(ps[64 * h:64 * h + 64, :], sm[wc],
                                         xq[f"{pref}c{h}"],
                                         start=True, stop=False, tile_position=tp)
                        nc.tensor.matmul(ps[64 * h:64 * h + 64, :], sm[wr],
                                         xq[f"{pref}s{h}"],
                                         start=False, stop=True, tile_position=tp)
                    nc.vector.tensor_copy(dst[:, cc:cc + CH], ps)
                return run

            steps.append(cmp_kcqa(kc_all[b], "wkc", "wkcr", "k", bankj(1)))
            steps.append(cmp_kcqa(qa_all[b], "wqa", "wqar", "q", bankj(0)))

            def cmp_qg(h, ps_tag):
                def run():
                    ps = psA(ps_tag)
                    nc.tensor.matmul(ps, sm["wqg"], xq[f"qc{h}"],
                                     start=True, stop=False)
                    nc.tensor.matmul(ps, sm["wqgr"], xq[f"qs{h}"],
                                     start=False, stop=True)
                    # 2*silu(x) = x * (1 + tanh(x/2)); 0.5 folded into wovp
                    tg = mpool.tile([128, CH], bf16, name="tg", tag="tg")
                    nc.scalar.activation(tg, ps, AF.Tanh, scale=0.5)
                    nc.vector.scalar_tensor_tensor(
                        sg[b][h][:, cc:cc + CH], tg, 1.0, ps,
                        ALU.add, ALU.mult)
                return run

            steps.append(cmp_qg(0, bankj(1)))
            steps.append(cmp_qg(1, bankj(0)))

            def vtrans():
                # vfold psum came out rank-major [128=2x64 ranks, CH tokens];
                # PE-transpose 128-token blocks to keys-major for the oc matmul
                ps = psum.tile([128, 4, 128], bf16, name="tV", tag=bankj(1))
                for tt in range(CH // 128):
                    nc.tensor.transpose(
                        ps[:, tt, :],
                        qkv_sb[4][:, tt * 128:(tt + 1) * 128],
                        sm["ident"],
                    )
                for h in range(H_LOCAL):
                    nc.vector.tensor_copy(
                        vc[b][h][:, 4 * c:4 * c + 4, 1:R1],
                        ps[:, :, 64 * h:64 * h + 64],
                    )
            steps.append(vtrans)

            if direct:
                for s in steps:
                    s()
            else:
                filler.extend(steps)

        # ---------------- Phase B q-chunk ----------------
        def emit_qcB(b, qc, oc_tags, wo_on_ocpair):
            base = b * S
            qcc = qc * CH
            oc_ps = [psum.tile([R1, CH], f32, name=t, tag=t) for t in oc_tags]
            wide = [ps_wide("wide0"), ps_wide("wide1")]

            def scores(kt):
                w = wide[kt % 2]
                for h in range(H_LOCAL):
                    nc.tensor.matmul(
                        w[:, h, :],
                        kc_all[b][64 * h:64 * h + 64, kt * 128:(kt + 1) * 128],
                        qa_all[b][64 * h:64 * h + 64, qcc:qcc + CH],
                        start=True, stop=True,
                        tile_position=(64 * h, 0),
                    )
                nc.scalar.activation(
                    exps[:, kt % NSLOT, :, :], w[:, :, :], AF.Exp,
                    scale=float(1.0 / np.sqrt(RANK)))

            def oc(kt):
                for h in range(H_LOCAL):
                    nc.tensor.matmul(
                        oc_ps[h],
                        vc[b][h][:, kt, :],
                        exps[:, kt % NSLOT, h, :],
                        start=(kt == 0),
                        stop=(kt == KT - 1),
                    )

            for kt in range(KT):
                scores(kt)
                if kt > 0:
                    oc(kt - 1)
                pump(1)
            oc(KT - 1)

            gated = []
            for h in range(H_LOCAL):
                # den/recip/broadcast chain first (longest serial path)
                den = mpool.tile([1, CH], f32, name=f"den{h}", tag=f"den{h}", bufs=1)
                nc.vector.tensor_copy(den, oc_ps[h][0:1, :])
                rdet = mpool.tile([1, CH], f32, name=f"rdet{h}", tag=f"rdet{h}", bufs=1)
                nc.vector.reciprocal_approx_fast(rdet, den)
                rdb = mpool.tile([1, CH], bf16, name=f"rdb{h}", tag=f"rdb{h}", bufs=1)
                nc.vector.tensor_copy(rdb, rdet)
                bcast = mpool.tile([128, CH], bf16, name=f"bc{h}", tag=f"bc{h}")
                nc.gpsimd.partition_broadcast(bcast, rdb)
                oc_sb = mpool.tile([R1, CH], bf16, name=f"ocsb{h}", tag=f"ocsb{h}")
                nc.vector.tensor_copy(oc_sb, oc_ps[h])
                # uplift on this qc's freed oc bank so wide frees right after
                # the last exp -> next qc's scores start immediately
                up = psum.tile([128, CH], f32, name=f"up{h}", tag=oc_tags[h])
                nc.tensor.matmul(up, sm["wovp"], oc_sb, start=True, stop=True)
                t1 = mpool.tile([128, CH], bf16, name=f"t1{h}", tag=f"t1{h}", bufs=1)
                nc.vector.tensor_mul(t1, up, sg[b][h][:, qcc:qcc + CH])
                g = mpool.tile([128, CH], bf16, name=f"gated{h}", tag=f"gated{h}")
                nc.vector.tensor_mul(g, t1, bcast)
                gated.append(g)

            # Wo never touches the wide banks: in the middle it self-paces on
            # cA (A-filler keeps to pA), in the tail on this qc's freed oc
            # pair — either way the next qc's kt loop runs concurrently
            wo_tags = oc_tags if wo_on_ocpair else ("cA",)
            wo_slices = [
                (lambda t=t: psum.tile([128, CH], f32, name=f"wo{t}", tag=t))
                for t in wo_tags]

            ngroups = (CH // 128) * (HIDDEN // CH)
            for gi in range(ngroups):
                ps = wo_slices[gi % len(wo_slices)]
                p = ps() if callable(ps) else ps
                for h in range(H_LOCAL):
                    nc.tensor.matmul(
                        p,
                        gated[h][:, (gi // (HIDDEN // CH)) * 128:
                                 (gi // (HIDDEN // CH)) * 128 + 128],
                        w_o[:, h, (gi % (HIDDEN // CH)) * CH:
                            (gi % (HIDDEN // CH) + 1) * CH],
                        start=(h == 0),
                        stop=(h == H_LOCAL - 1),
                    )
                ost = opool.tile([128, CH], bf16, name="ost", tag="ost")
                nc.vector.tensor_copy(ost, p)
                r0 = base + qcc + (gi // (HIDDEN // CH)) * 128
                nc.sync.dma_start(
                    out=out_d[r0:r0 + 128,
                              (gi % (HIDDEN // CH)) * CH:
                              (gi % (HIDDEN // CH) + 1) * CH],
                    in_=ost)

        # ---------------- emission schedule ----------------
        # startup: interleave wqkv p-slices with chunk 0's hidden p-slices so
        # the first j-chain matmul can start after ~0.3MB of DMA, not 5MB
        nc.vector.memset(cext[ROPE_DIM:128, :], 1.0)
        nc.vector.memset(sext[ROPE_DIM:128, :], 0.0)
        h_t0 = hpool.tile([128, PT, CH], bf16, name="hch", tag="hch")
        for p in range(PT):
            nc.sync.dma_start(out=w_qkv[:, p, :], in_=wqkv[p])
            nc.sync.dma_start(out=h_t0[:, p, :],
                              in_=ht[p * 128:(p + 1) * 128, 0:CH])
        nc.sync.dma_start(out=cext[0:ROPE_DIM, :], in_=cext_d[:])
        nc.sync.dma_start(out=sext[0:ROPE_DIM, :], in_=sext_d[:])
        h_t1 = chunk_dma(0, 1)
        emit_chunkA(0, 0, direct=True, h_t=h_t0)
        emit_chunkA(0, 1, direct=True, h_t=h_t1)
        for c in range(2, NCH):
            emit_chunkA(0, c, direct=True)
        for h in range(H_LOCAL):
            nc.sync.dma_start(out=w_o[:, h, :], in_=wo_d[h])
        for i in range(QC):
            emit_chunkA(1, i, direct=False)
            emit_qcB(0, i, ("oc0", "oc1"), wo_on_ocpair=False)
            flush()
        for i in range(QC):
            tags = ("oc0", "oc1") if i % 2 == 0 else ("pA", "cA")
            emit_qcB(1, i, tags, wo_on_ocpair=True)
        flush()

    nc.compile()
    return nc


def _rot_w(w):
    """Fold rotate-half into a projection matrix (see module docstring)."""
    r = np.zeros_like(w)
    r[0:32] = w[32:64]
    r[32:64] = -w[0:32]
    return r


def _host_inputs(hidden_states, position_ids, Wq, Wk, Wv, Wkc, Wvc, Wqa, Wqg,
                 Wov, Wo, S):
    """Build the 8 per-core input maps (all device arrays bf16)."""
    B = 2
    T = B * S
    h = np.asarray(hidden_states, dtype=np.float32).reshape(T, HIDDEN)
    ht = np.ascontiguousarray(h.T).astype(BF16)

    pos = np.asarray(position_ids).reshape(-1).astype(np.float64)
    pos = np.concatenate([pos] * B)  # token order is [b0 tokens, b1 tokens]
    inv_freq = 1.0 / (BASE ** (np.arange(0, ROPE_DIM, 2, dtype=np.float64) / ROPE_DIM))
    freqs = np.outer(pos, inv_freq)                       # [T, 32]
    emb = np.concatenate([freqs, freqs], axis=1)          # [T, 64]
    cext = np.ascontiguousarray(np.cos(emb).T.astype(np.float32))  # [64, T]
    sext = np.ascontiguousarray(np.sin(emb).T.astype(np.float32))

    Wkc = np.asarray(Wkc, np.float32); Wvc = np.asarray(Wvc, np.float32)
    Wqa = np.asarray(Wqa, np.float32); Wqg = np.asarray(Wqg, np.float32)
    Wov = np.asarray(Wov, np.float32)
    # 0.5 factor: kernel computes 2*silu via the tanh identity
    wovp = np.concatenate([np.zeros((1, 128), np.float32), 0.5 * Wov], axis=0)

    shared = {
        "ht": ht,
        "wkc": Wkc.astype(BF16), "wkcr": _rot_w(Wkc).astype(BF16),
        "wqa": Wqa.astype(BF16), "wqar": _rot_w(Wqa).astype(BF16),
        "wqg": Wqg.astype(BF16), "wqgr": _rot_w(Wqg).astype(BF16),
        "wovp": wovp.astype(BF16),
        "ident": np.eye(128, dtype=np.float32).astype(BF16),
        "cext": cext.astype(BF16), "sext": sext.astype(BF16),
    }

    Wq = np.asarray(Wq, np.float32); Wk = np.asarray(Wk, np.float32)
    Wv = np.asarray(Wv, np.float32); Wo = np.asarray(Wo, np.float32)
    in_maps = []
    for c in range(N_CORES):
        cols = slice(c * 256, (c + 1) * 256)
        vf = [Wv[:, c * 256 + 128 * h:c * 256 + 128 * (h + 1)] @ Wvc
              for h in range(H_LOCAL)]                     # each [2048, 64]
        wbig = np.concatenate(
            [Wq[:, cols], Wk[:, cols]] + vf, axis=1)       # [2048, 640]
        m = dict(shared)
        m["wqkv"] = np.ascontiguousarray(
            wbig.reshape(HIDDEN // 128, 128, 640)).astype(BF16)
        m["wo"] = np.ascontiguousarray(
            Wo[cols].reshape(H_LOCAL, 128, HIDDEN)).astype(BF16)
        in_maps.append(m)
    return in_maps


_NC_CACHE = {}


def kernel(hidden_states, position_ids, Wq, Wk, Wv, Wkc, Wvc, Wqa, Wqg, Wov,
           Wo, _trace=False):
    from concourse.bass_utils import run_bass_kernel_spmd

    B, S, _ = np.asarray(hidden_states).shape
    assert B == 2
    in_maps = _host_inputs(hidden_states, position_ids, Wq, Wk, Wv, Wkc, Wvc,
                           Wqa, Wqg, Wov, Wo, S)
    if S not in _NC_CACHE:
        _NC_CACHE[S] = _build_nc(S)
    nc = _NC_CACHE[S]
    res = run_bass_kernel_spmd(nc, in_maps, list(range(N_CORES)), trace=_trace)
    out = np.zeros((B * S, HIDDEN), dtype=np.float32)
    for r in res.results:
        out += np.asarray(r["out"]).astype(np.float32)
    kernel.last_results = res
    return out.reshape(B, S, HIDDEN)


# revision 25
# speedup vs baseline: 1.2078x; 1.0139x over previous
"""DeepSeek-MLA attention block on 8 Trainium2 NeuronCores.

Sharding: tensor-parallel over heads (16 heads / 8 cores = 2 heads per core).
All per-head projections (k/v compressor, q_a, q_gate, o_v) are head-local;
each core computes a partial output through its slice of Wo and the host sums
the 8 bf16 partials in fp32.

Device layout notes:
  - Activations live transposed: [d, token] tiles so the d_model contraction
    sits on the partition dim.
  - RoPE is folded into the compression matmuls: for a per-token rotation
    x' = c .* x + s .* (P x) (P the fixed rotate-half permutation), any
    projection W satisfies  x'^T W = (c .* x)^T W + (s .* x)^T W_rot  with
    W_rot[e,:] = W[e+32,:] (e<32), -W[e-32,:] (32<=e<64), 0 (e>=64).
  - v has no RoPE, so Wv @ Wvc is folded host-side into a single 2048->64
    per-head projection computed inside the big QKV matmul (640 cols instead
    of 768).  The folded result comes out rank-major; 4 PE-transposes per
    chunk flip it to the keys-major layout the attention matmul needs.
  - Softmax runs unnormalized: exp(scores/8) with the denominator obtained by
    prepending a ones-column to v_c (row 0 of the out_c accumulation is then
    sum_k exp; a zero row in the padded Wov cancels it in the uplift).
  - silu is computed via tanh (same ACT table set as exp, avoiding 1.3us
    activation-table reloads when phase A and B interleave):
    2*silu(x) = x*(1+tanh(x/2)); the 0.5 is folded into Wov host-side.
  - Each kt step's two heads' scores go into one [128,2,512] 2-bank psum
    tile so a single ACT instruction computes exp for both heads (1024 wide).
  - The two batches' phases are software-pipelined: A(b0); then 4 units of
    [B(b0,qc_i) kt-loop with A(b1,chunk_i) matmul-chain steps woven into the
    ACT-bound kt slots]; then B(b1) with the previous qc's Wo groups woven in.
  - PSUM (8 banks): pA,cA = phase-A chains (+ deferred Wo in the tail),
    wide0,wide1 = scores/exp (2 banks each, also uplift + middle Wo slices),
    oc0,oc1 = out_c accumulators.
"""

import collections
import numpy as np
import ml_dtypes

HIDDEN = 2048
N_HEADS = 16
HEAD_DIM = 128
ROPE_DIM = 64
RANK = 64
BASE = 10000.0
N_CORES = 8
H_LOCAL = 2  # heads per core

BF16 = ml_dtypes.bfloat16


def _build_nc(S: int, CH: int = 512):
    """Build the SPMD Bass program for one core (B=2 fixed, seq len S)."""
    import concourse.bacc as bacc
    import concourse.tile as tile
    from concourse import mybir
    from contextlib import ExitStack

    f32 = mybir.dt.float32
    bf16 = mybir.dt.bfloat16
    AF = mybir.ActivationFunctionType
    ALU = mybir.AluOpType

    B = 2
    T = B * S
    PT = HIDDEN // 128          # 16 d_model partition tiles
    KT = S // 128               # 16 key tiles per batch
    NCH = S // CH               # 4 token chunks per batch (phase A)
    QC = S // CH                # 4 q chunks per batch (phase B)
    R1 = RANK + 1               # out_c rows incl. denominator row 0
    NJ = 5                      # qkv col groups: q0,q1,k0,k1,vfold

    nc = bacc.Bacc("TRN2", target_bir_lowering=False, debug=False)

    ht = nc.dram_tensor("ht", [HIDDEN, T], bf16, kind="ExternalInput")
    wqkv = nc.dram_tensor("wqkv", [PT, 128, NJ * 128], bf16, kind="ExternalInput")
    wo_d = nc.dram_tensor("wo", [H_LOCAL, 128, HIDDEN], bf16, kind="ExternalInput")
    wkc_d = nc.dram_tensor("wkc", [128, RANK], bf16, kind="ExternalInput")
    wkcr_d = nc.dram_tensor("wkcr", [128, RANK], bf16, kind="ExternalInput")
    wqa_d = nc.dram_tensor("wqa", [128, RANK], bf16, kind="ExternalInput")
    wqar_d = nc.dram_tensor("wqar", [128, RANK], bf16, kind="ExternalInput")
    wqg_d = nc.dram_tensor("wqg", [128, 128], bf16, kind="ExternalInput")
    wqgr_d = nc.dram_tensor("wqgr", [128, 128], bf16, kind="ExternalInput")
    wovp_d = nc.dram_tensor("wovp", [R1, 128], bf16, kind="ExternalInput")
    ident_d = nc.dram_tensor("ident", [128, 128], bf16, kind="ExternalInput")
    # only the 64 rope rows carry data; rows 64-127 are memset on device
    cext_d = nc.dram_tensor("cext", [ROPE_DIM, T], bf16, kind="ExternalInput")
    sext_d = nc.dram_tensor("sext", [ROPE_DIM, T], bf16, kind="ExternalInput")
    out_d = nc.dram_tensor("out", [T, HIDDEN], bf16, kind="ExternalOutput")

    with tile.TileContext(nc) as tc, ExitStack() as ctx:
        const = ctx.enter_context(tc.tile_pool(name="const", bufs=1))
        hpool = ctx.enter_context(tc.tile_pool(name="hp", bufs=2))
        qkvpool = ctx.enter_context(tc.tile_pool(name="qkvp", bufs=2))
        xpool = ctx.enter_context(tc.tile_pool(name="xp", bufs=1))
        cpool = ctx.enter_context(tc.tile_pool(name="cp", bufs=1))
        epool = ctx.enter_context(tc.tile_pool(name="ep", bufs=1))
        mpool = ctx.enter_context(tc.tile_pool(name="mp", bufs=2))
        opool = ctx.enter_context(tc.tile_pool(name="op", bufs=4))
        psum = ctx.enter_context(tc.tile_pool(name="ps", bufs=1, space="PSUM"))

        # ---- psum bank layout: pin allocation order ----
        # pA(1) cA(1) wide0(2) wide1(2) oc0(1) oc1(1) = 8 banks
        def psA(tag):
            return psum.tile([128, CH], f32, name=tag, tag=tag)

        def ps_wide(tag):
            return psum.tile([128, 2, CH], f32, name=tag, tag=tag)

        def ps_oc(tag):
            return psum.tile([R1, CH], f32, name=tag, tag=tag)

        _pin = [psA("pA"), psA("cA"), ps_wide("wide0"), ps_wide("wide1"),
                ps_oc("oc0"), ps_oc("oc1")]
        del _pin

        # ---- constants ----
        # DMA order matters for startup latency: wqkv p-slices first (first
        # j-chain needs p=0 almost immediately), small weights next, cext /
        # sext after the first hidden chunk, w_o (needed only ~80us in) last.
        w_qkv = const.tile([128, PT, NJ * 128], bf16, name="wqkv", tag="wqkv")
        sm = {}
        small_dmas = []
        for name, d, shp in [
            ("wkc", wkc_d, [128, RANK]), ("wkcr", wkcr_d, [128, RANK]),
            ("wqa", wqa_d, [128, RANK]), ("wqar", wqar_d, [128, RANK]),
            ("wqg", wqg_d, [128, 128]), ("wqgr", wqgr_d, [128, 128]),
            ("wovp", wovp_d, [R1, 128]), ("ident", ident_d, [128, 128]),
        ]:
            t = const.tile(shp, bf16, name=name, tag=name)
            small_dmas.append((t, d))
            sm[name] = t
        cext = const.tile([128, T], bf16, name="cext", tag="cext")
        sext = const.tile([128, T], bf16, name="sext", tag="sext")
        w_o = const.tile([128, H_LOCAL, HIDDEN], bf16, name="wo", tag="wo")
        from concourse import library_config
        nc.gpsimd.load_library(library_config.attn)

        # ---- persistent per-batch state ----
        kc_all, qa_all, sg, vc = {}, {}, {}, {}
        for b in range(B):
            kc_all[b] = cpool.tile([128, S], bf16, name=f"kca{b}", tag=f"kca{b}")
            qa_all[b] = cpool.tile([128, S], bf16, name=f"qaa{b}", tag=f"qaa{b}")
            sg[b] = [cpool.tile([128, S], bf16, name=f"sg{b}{h}", tag=f"sg{b}{h}")
                     for h in range(H_LOCAL)]
            vc[b] = [cpool.tile([128, KT, R1], bf16, name=f"vc{b}{h}", tag=f"vc{b}{h}")
                     for h in range(H_LOCAL)]
            for h in range(H_LOCAL):
                nc.vector.memset(vc[b][h][:, :, 0:1], 1.0)
        NSLOT = 8   # exp lookahead slots: lets ACT run ahead of the oc chain
        exps = epool.tile([128, NSLOT, H_LOCAL, CH], bf16, name="exps", tag="exps")

        # ---- filler machinery: each step is ~2 matmuls of PE work ----
        filler = collections.deque()

        def pump(n=1):
            for _ in range(n):
                if not filler:
                    return
                filler.popleft()()

        def flush():
            while filler:
                filler.popleft()()

        # ---------------- Phase A chunk (emitted as steps) ----------------
        def chunk_dma(b, c):
            tcol = b * S + c * CH
            h_t = hpool.tile([128, PT, CH], bf16, name="hch", tag="hch")
            for p in range(PT):
                nc.sync.dma_start(
                    out=h_t[:, p, :],
                    in_=ht[p * 128:(p + 1) * 128, tcol:tcol + CH],
                )
            return h_t

        def emit_chunkA(b, c, direct, h_t=None):
            base = b * S
            tcol = base + c * CH
            cc = c * CH
            if h_t is None:
                h_t = chunk_dma(b, c)
            # direct (front) chunks ping-pong 2 banks for max PE rate;
            # filler (middle) chunks stay on pA only, leaving cA for the
            # concurrent Wo groups of the interleaved B q-chunks
            bankj = (lambda j: "pA" if j % 2 == 0 else "cA") if direct \
                else (lambda j: "pA")
            csl = cext[:, tcol:tcol + CH]
            ssl = sext[:, tcol:tcol + CH]
            steps = []
            qkv_sb = []
            xq = {}  # rope-multiplied tiles

            def j_chain(j, ps_tag):
                ps = psA(ps_tag)

                def step(k, ps=ps, j=j):
                    def run():
                        for p in (2 * k, 2 * k + 1):
                            nc.tensor.matmul(
                                ps,
                                w_qkv[:, p, j * 128:(j + 1) * 128],
                                h_t[:, p, :],
                                start=(p == 0),
                                stop=(p == PT - 1),
                            )
                        if 2 * k + 1 == PT - 1:
                            sb = qkvpool.tile([128, CH], bf16,
                                              name=f"qsb{j}", tag=f"qsb{j}")
                            nc.vector.tensor_copy(sb, ps)
                            qkv_sb.append(sb)
                            if j == 1:      # q tiles drained -> rope muls
                                for h in range(H_LOCAL):
                                    for nm, sl in (("qc", csl), ("qs", ssl)):
                                        t = xpool.tile(
                                            [128, CH], bf16,
                                            name=f"x{nm}{h}", tag=f"x{nm}{h}")
                                        nc.vector.tensor_mul(t, qkv_sb[h], sl)
                                        xq[f"{nm}{h}"] = t
                            if j == 3:      # k tiles drained
                                for h in range(H_LOCAL):
                                    for nm, sl in (("kc", csl), ("ks", ssl)):
                                        t = xpool.tile(
                                            [128, CH], bf16,
                                            name=f"x{nm}{h}", tag=f"x{nm}{h}")
                                        nc.vector.tensor_mul(t, qkv_sb[2 + h], sl)
                                        xq[f"{nm}{h}"] = t
                    return run
                return [step(k) for k in range(PT // 2)]

            for j in range(NJ):
                steps += j_chain(j, bankj(j))

            def cmp_kcqa(dst, wc, wr, pref, ps_tag):
                def run():
                    ps = psA(ps_tag)
                    for h in range(H_LOCAL):
                        tp = (0, 64 * h) if h else None
                        nc.tensor.matmul(ps[64 * h:64 * h + 64, :], sm[wc],
                                         xq[f"{pref}c{h}"],
                                         start=True, stop=False, tile_position=tp)
                        nc.tensor.matmul(ps[64 * h:64 * h + 64, :], sm[wr],
                                         xq[f"{pref}s{h}"],
                                         start=False, stop=True, tile_position=tp)
                    nc.vector.tensor_copy(dst[:, cc:cc + CH], ps)
                return run

            steps.append(cmp_kcqa(kc_all[b], "wkc", "wkcr", "k", bankj(1)))
            steps.append(cmp_kcqa(qa_all[b], "wqa", "wqar", "q", bankj(0)))

            def cmp_qg(h, ps_tag):
                def run():
                    ps = psA(ps_tag)
                    nc.tensor.matmul(ps, sm["wqg"], xq[f"qc{h}"],
                                     start=True, stop=False)
                    nc.tensor.matmul(ps, sm["wqgr"], xq[f"qs{h}"],
                                     start=False, stop=True)
                    # 2*silu(x) = x * (1 + tanh(x/2)); 0.5 folded into wovp
                    tg = mpool.tile([128, CH], bf16, name="tg", tag="tg")
                    nc.scalar.activation(tg, ps, AF.Tanh, scale=0.5)
                    nc.vector.scalar_tensor_tensor(
                        sg[b][h][:, cc:cc + CH], tg, 1.0, ps,
                        ALU.add, ALU.mult)
                return run

            steps.append(cmp_qg(0, bankj(1)))
            steps.append(cmp_qg(1, bankj(0)))

            def vtrans():
                # vfold psum came out rank-major [128=2x64 ranks, CH tokens];
                # PE-transpose 128-token blocks to keys-major for the oc matmul
                ps = psum.tile([128, 4, 128], bf16, name="tV", tag=bankj(1))
                for tt in range(CH // 128):
                    nc.tensor.transpose(
                        ps[:, tt, :],
                        qkv_sb[4][:, tt * 128:(tt + 1) * 128],
                        sm["ident"],
                    )
                for h in range(H_LOCAL):
                    nc.vector.tensor_copy(
                        vc[b][h][:, 4 * c:4 * c + 4, 1:R1],
                        ps[:, :, 64 * h:64 * h + 64],
                    )
            steps.append(vtrans)

            if direct:
                for s in steps:
                    s()
            else:
                filler.extend(steps)

        # ---------------- Phase B q-chunk ----------------
        def emit_qcB(b, qc, oc_tags, wo_on_ocpair):
            base = b * S
            qcc = qc * CH
            oc_ps = [psum.tile([R1, CH], f32, name=t, tag=t) for t in oc_tags]
            wide = [ps_wide("wide0"), ps_wide("wide1")]

            def scores(kt):
                w = wide[kt % 2]
                for h in range(H_LOCAL):
                    nc.tensor.matmul(
                        w[:, h, :],
                        kc_all[b][64 * h:64 * h + 64, kt * 128:(kt + 1) * 128],
                        qa_all[b][64 * h:64 * h + 64, qcc:qcc + CH],
                        start=True, stop=True,
                        tile_position=(64 * h, 0),
                    )
                nc.scalar.activation(
                    exps[:, kt % NSLOT, :, :], w[:, :, :], AF.Exp,
                    scale=float(1.0 / np.sqrt(RANK)))

            def oc(kt):
                for h in range(H_LOCAL):
                    nc.tensor.matmul(
                        oc_ps[h],
                        vc[b][h][:, kt, :],
                        exps[:, kt % NSLOT, h, :],
                        start=(kt == 0),
                        stop=(kt == KT - 1),
                    )

            for kt in range(KT):
                scores(kt)
                if kt > 0:
                    oc(kt - 1)
                pump(1)
            oc(KT - 1)

            gated = []
            for h in range(H_LOCAL):
                # den/recip/broadcast chain first (longest serial path)
                den = mpool.tile([1, CH], f32, name=f"den{h}", tag=f"den{h}", bufs=1)
                nc.vector.tensor_copy(den, oc_ps[h][0:1, :])
                rdet = mpool.tile([1, CH], f32, name=f"rdet{h}", tag=f"rdet{h}", bufs=1)
                nc.vector.reciprocal_approx_fast(rdet, den)
                rdb = mpool.tile([1, CH], bf16, name=f"rdb{h}", tag=f"rdb{h}", bufs=1)
                nc.vector.tensor_copy(rdb, rdet)
                bcast = mpool.tile([128, CH], bf16, name=f"bc{h}", tag=f"bc{h}")
                nc.gpsimd.partition_broadcast(bcast, rdb)
                oc_sb = mpool.tile([R1, CH], bf16, name=f"ocsb{h}", tag=f"ocsb{h}")
                nc.vector.tensor_copy(oc_sb, oc_ps[h])
                # uplift on this qc's freed oc bank so wide frees right after
                # the last exp -> next qc's scores start immediately
                up = psum.tile([128, CH], f32, name=f"up{h}", tag=oc_tags[h])
                nc.tensor.matmul(up, sm["wovp"], oc_sb, start=True, stop=True)
                t1 = mpool.tile([128, CH], bf16, name=f"t1{h}", tag=f"t1{h}", bufs=1)
                nc.vector.tensor_mul(t1, up, sg[b][h][:, qcc:qcc + CH])
                g = mpool.tile([128, CH], bf16, name=f"gated{h}", tag=f"gated{h}")
                nc.vector.tensor_mul(g, t1, bcast)
                gated.append(g)

            # Wo never touches the wide banks: in the middle it self-paces on
            # cA (A-filler keeps to pA), in the tail on this qc's freed oc
            # pair — either way the next qc's kt loop runs concurrently
            wo_tags = oc_tags if wo_on_ocpair else ("cA",)
            wo_slices = [
                (lambda t=t: psum.tile([128, CH], f32, name=f"wo{t}", tag=t))
                for t in wo_tags]

            NG = HIDDEN // CH
            for gi in range((CH // 128) * NG):
                tt, n = divmod(gi, NG)
                ps = wo_slices[gi % len(wo_slices)]
                p = ps() if callable(ps) else ps
                for h in range(H_LOCAL):
                    nc.tensor.matmul(
                        p,
                        gated[h][:, tt * 128:tt * 128 + 128],
                        w_o[:, h, n * CH:(n + 1) * CH],
                        start=(h == 0),
                        stop=(h == H_LOCAL - 1),
                    )
                if n == 0:
                    ost = opool.tile([128, NG, CH], bf16, name="ost",
                                     tag="ost", bufs=2)
                nc.vector.tensor_copy(ost[:, n, :], p)
                if n == NG - 1:
                    # one batched row-DMA per token tile (fewer sync issues)
                    r0 = base + qcc + tt * 128
                    nc.sync.dma_start(out=out_d[r0:r0 + 128, :], in_=ost)

        # ---------------- emission schedule ----------------
        # startup: interleave wqkv p-slices with chunk 0's hidden p-slices so
        # the first j-chain matmul can start after ~0.3MB of DMA, not 5MB
        nc.vector.memset(cext[ROPE_DIM:128, :], 1.0)
        nc.vector.memset(sext[ROPE_DIM:128, :], 0.0)
        h_t0 = hpool.tile([128, PT, CH], bf16, name="hch", tag="hch")
        for p in range(PT):
            nc.sync.dma_start(out=w_qkv[:, p, :], in_=wqkv[p])
            nc.sync.dma_start(out=h_t0[:, p, :],
                              in_=ht[p * 128:(p + 1) * 128, 0:CH])
        for t, dr in small_dmas:
            nc.sync.dma_start(out=t[:], in_=dr[:])
        nc.sync.dma_start(out=cext[0:ROPE_DIM, :], in_=cext_d[:])
        nc.sync.dma_start(out=sext[0:ROPE_DIM, :], in_=sext_d[:])
        h_t1 = chunk_dma(0, 1)
        emit_chunkA(0, 0, direct=True, h_t=h_t0)
        emit_chunkA(0, 1, direct=True, h_t=h_t1)
        for c in range(2, NCH):
            emit_chunkA(0, c, direct=True)
        for h in range(H_LOCAL):
            nc.sync.dma_start(out=w_o[:, h, :], in_=wo_d[h])
        for i in range(QC):
            emit_chunkA(1, i, direct=False)
            emit_qcB(0, i, ("oc0", "oc1"), wo_on_ocpair=False)
            flush()
        for i in range(QC):
            tags = ("oc0", "oc1") if i % 2 == 0 else ("pA", "cA")
            emit_qcB(1, i, tags, wo_on_ocpair=True)
        flush()

    nc.compile()
    return nc


def _rot_w(w):
    """Fold rotate-half into a projection matrix (see module docstring)."""
    r = np.zeros_like(w)
    r[0:32] = w[32:64]
    r[32:64] = -w[0:32]
    return r


def _host_inputs(hidden_states, position_ids, Wq, Wk, Wv, Wkc, Wvc, Wqa, Wqg,
                 Wov, Wo, S):
    """Build the 8 per-core input maps (all device arrays bf16)."""
    B = 2
    T = B * S
    h = np.asarray(hidden_states, dtype=np.float32).reshape(T, HIDDEN)
    ht = np.ascontiguousarray(h.T).astype(BF16)

    pos = np.asarray(position_ids).reshape(-1).astype(np.float64)
    pos = np.concatenate([pos] * B)  # token order is [b0 tokens, b1 tokens]
    inv_freq = 1.0 / (BASE ** (np.arange(0, ROPE_DIM, 2, dtype=np.float64) / ROPE_DIM))
    freqs = np.outer(pos, inv_freq)                       # [T, 32]
    emb = np.concatenate([freqs, freqs], axis=1)          # [T, 64]
    cext = np.ascontiguousarray(np.cos(emb).T.astype(np.float32))  # [64, T]
    sext = np.ascontiguousarray(np.sin(emb).T.astype(np.float32))

    Wkc = np.asarray(Wkc, np.float32); Wvc = np.asarray(Wvc, np.float32)
    Wqa = np.asarray(Wqa, np.float32); Wqg = np.asarray(Wqg, np.float32)
    Wov = np.asarray(Wov, np.float32)
    # 0.5 factor: kernel computes 2*silu via the tanh identity
    wovp = np.concatenate([np.zeros((1, 128), np.float32), 0.5 * Wov], axis=0)

    shared = {
        "ht": ht,
        "wkc": Wkc.astype(BF16), "wkcr": _rot_w(Wkc).astype(BF16),
        "wqa": Wqa.astype(BF16), "wqar": _rot_w(Wqa).astype(BF16),
        "wqg": Wqg.astype(BF16), "wqgr": _rot_w(Wqg).astype(BF16),
        "wovp": wovp.astype(BF16),
        "ident": np.eye(128, dtype=np.float32).astype(BF16),
        "cext": cext.astype(BF16), "sext": sext.astype(BF16),
    }

    Wq = np.asarray(Wq, np.float32); Wk = np.asarray(Wk, np.float32)
    Wv = np.asarray(Wv, np.float32); Wo = np.asarray(Wo, np.float32)
    in_maps = []
    for c in range(N_CORES):
        cols = slice(c * 256, (c + 1) * 256)
        vf = [Wv[:, c * 256 + 128 * h:c * 256 + 128 * (h + 1)] @ Wvc
              for h in range(H_LOCAL)]                     # each [2048, 64]
        wbig = np.concatenate(
            [Wq[:, cols], Wk[:, cols]] + vf, axis=1)       # [2048, 640]
        m = dict(shared)
        m["wqkv"] = np.ascontiguousarray(
            wbig.reshape(HIDDEN // 128, 128, 640)).astype(BF16)
        m["wo"] = np.ascontiguousarray(
            Wo[cols].reshape(H_LOCAL, 128, HIDDEN)).astype(BF16)
        in_maps.append(m)
    return in_maps


_NC_CACHE = {}


def kernel(hidden_states, position_ids, Wq, Wk, Wv, Wkc, Wvc, Wqa, Wqg, Wov,
           Wo, _trace=False):
    from concourse.bass_utils import run_bass_kernel_spmd

    B, S, _ = np.asarray(hidden_states).shape
    assert B == 2
    in_maps = _host_inputs(hidden_states, position_ids, Wq, Wk, Wv, Wkc, Wvc,
                           Wqa, Wqg, Wov, Wo, S)
    if S not in _NC_CACHE:
        _NC_CACHE[S] = _build_nc(S)
    nc = _NC_CACHE[S]
    res = run_bass_kernel_spmd(nc, in_maps, list(range(N_CORES)), trace=_trace)
    out = np.zeros((B * S, HIDDEN), dtype=np.float32)
    for r in res.results:
        out += np.asarray(r["out"]).astype(np.float32)
    kernel.last_results = res
    return out.reshape(B, S, HIDDEN)
